# revision 28
# baseline (speedup 1.0000x reference)
"""Causal self-attention on 8 Trainium2 NeuronCores.

Problem (hardcoded): B=4, T=2048, C=1024, H=16, D=64.
  qkv = x @ w_qkv + b_qkv ; per-head causal softmax attention ; out = attn @ w_proj + b_proj

Sharding (per hint): tensor-parallel over heads x data-parallel over batch.
  core c -> batch b = c // 2, head group g = c % 2 (heads g*8 .. g*8+7).
Each core computes QKV for its 8 heads, causal attention, and a partial
projection (its 512 input channels of w_proj). Host sums the two partials per
batch and adds b_proj.

v2: virtual-clock emitter. Attention (S -> exp -> PV) is paced by the Scalar
engine's exp; all other PE work (QKV projections, output projection) is
emitted matmul-granular into the predicted exp-stall windows from a dedicated
2-bank PSUM fill pool, so fills never wait on the exp-paced S ring (which
shrinks to 2 slots). Vector load is cut by merging the softmax-denominator
row into the O-tile drain (one copy instead of two), reciprocal in place, and
moving the normalize broadcast+multiply to the (otherwise idle) GpSimd/Pool
engine. PE warmup matmuls run at t=0 against a memset dummy tile so the
p-state ramp happens before real work; first DMA parcels are split so the
initial transfers fan across multiple HW queues.
"""

import numpy as np
import ml_dtypes

B, T, C, H, D = 4, 2048, 1024, 16, 64
HL = H // 2          # heads per core
CL = HL * D          # local channels (512)
NPAIR = HL // 2      # head pairs per core (4)
CCH = C // 128       # contraction chunks for qkv (8)
PCH = CL // 128      # contraction chunks for proj (4)
TT = T // 128        # t tiles (16)
NI = T // 512        # i chunks (4)
N_CORES = 8
BF16 = ml_dtypes.bfloat16

# ---- virtual-clock cost constants (ns) ----
MM = 217.0        # N=512 matmul issue slot
SEM = 150.0       # cross-engine semaphore hop
EXP_FIX = 190.0   # activation fixed overhead
EXP_EL = 1.0 / 1.2
MASK_C = 330.0
OC_C = 700.0
RECIP_C = 560.0
DRAIN_C = 800.0   # [128,512] psum->sbuf drain on DVE
BCAST_C = 650.0
PMUL_C = 1150.0
DMA_BW = 0.0029   # ns per byte (~345 GB/s aggregate)

N_WARMUP = 24     # dummy PE warmup matmuls

_compiled = {}


def _build(nc, bias_zero=False):
    import concourse.tile as tile
    from concourse import mybir

    bf = mybir.dt.bfloat16
    f32 = mybir.dt.float32
    Exp = mybir.ActivationFunctionType.Exp
    Ident = mybir.ActivationFunctionType.Identity

    xT = nc.dram_tensor("xT", [128, 4 * CCH * 512], bf, kind="ExternalInput").ap()
    wq = nc.dram_tensor("wq", [NPAIR, 128, CCH * 128], bf, kind="ExternalInput").ap()
    wk = nc.dram_tensor("wk", [NPAIR, 128, CCH * 128], bf, kind="ExternalInput").ap()
    wv = nc.dram_tensor("wv", [128, CCH * CL], bf, kind="ExternalInput").ap()
    bq = nc.dram_tensor("bq", [128, NPAIR], f32, kind="ExternalInput").ap()
    bk = nc.dram_tensor("bk", [128, NPAIR], f32, kind="ExternalInput").ap()
    bv = nc.dram_tensor("bv", [1, CL], f32, kind="ExternalInput").ap()
    wp = nc.dram_tensor("wp", [CL, C], bf, kind="ExternalInput").ap()
    out = nc.dram_tensor("out", [T, C], bf, kind="ExternalOutput").ap()

    xT_r = xT.rearrange("p (q cc t) -> p q cc t", q=4, cc=CCH)
    wv_r = wv.rearrange("p (cc m) -> p cc m", cc=CCH)
    wp_r = wp.rearrange("(cc p) n -> p cc n", p=128)
    wq_r = wq.rearrange("a p (cc m) -> a p cc m", m=128)
    wk_r = wk.rearrange("a p (cc m) -> a p cc m", m=128)

    with tile.TileContext(nc) as tc:
        import contextlib

        with contextlib.ExitStack() as ctx:
            persist = ctx.enter_context(tc.tile_pool(name="persist", bufs=1))
            # PSUM: s-ring 2x[128,2,512] (4 banks) + o0/o1 (2 banks) +
            # fill pool 2x[128,512] (2 banks) = 8 banks.
            ps_pool = ctx.enter_context(tc.tile_pool(name="ps_pool", bufs=2, space="PSUM"))
            o_ps = ctx.enter_context(tc.tile_pool(name="o_ps", bufs=1, space="PSUM"))
            f_ps = ctx.enter_context(tc.tile_pool(name="f_ps", bufs=2, space="PSUM"))
            p_pool = ctx.enter_context(tc.tile_pool(name="p_pool", bufs=8))
            oc_pool = ctx.enter_context(tc.tile_pool(name="oc_pool", bufs=2))
            rb_pool = ctx.enter_context(tc.tile_pool(name="rb_pool", bufs=2))
            st_pool = ctx.enter_context(tc.tile_pool(name="st_pool", bufs=4))

            # ---- persistent SBUF tensors ----
            xT_sb = persist.tile([128, 4, CCH, 512], bf)
            wq_sb = persist.tile([128, NPAIR, CCH, 128], bf)
            wk_sb = persist.tile([128, NPAIR, CCH, 128], bf)
            wv_sb = persist.tile([128, CCH, CL], bf)
            wp_sb = persist.tile([128, PCH, C], bf)
            bq_sb = persist.tile([128, NPAIR], f32)
            bk_sb = persist.tile([128, NPAIR], f32)
            bv1_sb = persist.tile([1, CL], f32)
            bv_sb = persist.tile([128, CL], f32)
            QT_sb = persist.tile([128, NPAIR, T], bf)
            KT_sb = persist.tile([128, NPAIR, T], bf)
            V_sb = persist.tile([128, TT, HL, 2 * D], bf)
            AT_sb = persist.tile([128, PCH, T], bf)
            tri_sb = persist.tile([128, 2, 128], bf)
            dum_sb = persist.tile([128, 512], bf)

            # ---- DMA emission (sync queue), first parcels split fine ----
            dma_t = [3500.0]
            arrive = {}

            def dstart(name, out_ap, in_ap, nbytes):
                nc.sync.dma_start(out=out_ap, in_=in_ap)
                dma_t[0] = dma_t[0] + 565.0
                t = max(dma_t[0] + 1500.0, arrive.get("_last", 0.0)) + nbytes * DMA_BW
                arrive[name] = t + 1200.0
                arrive["_last"] = t

            dstart("bv", bv1_sb[:], bv[:], 1024)
            # first wave: fan the first big parcels across the idle engine
            # queues so their HWDGE issues run concurrently
            dstart("x00", xT_sb[:, 0, 0:2], xT_r[:, 0, 0:2], 262144)
            nc.scalar.dma_start(out=wv_sb[:, 0:2], in_=wv_r[:, 0:2])
            nc.gpsimd.dma_start(out=wv_sb[:, 2:4], in_=wv_r[:, 2:4])
            arrive["wv0"] = 5600.0
            arrive["wv1"] = 6400.0
            for h in range(1, 4):
                if h > 1:
                    dstart(f"wv{h}", wv_sb[:, 2 * h : 2 * h + 2], wv_r[:, 2 * h : 2 * h + 2], 262144)
                dstart(f"x0{h}", xT_sb[:, 0, 2 * h : 2 * h + 2], xT_r[:, 0, 2 * h : 2 * h + 2], 262144)
            for p in range(NPAIR):
                dstart(f"wk{p}", wk_sb[:, p], wk_r[p], 262144)
                dstart(f"wq{p}", wq_sb[:, p], wq_r[p], 262144)
            dstart("bq", bq_sb[:], bq[:], 2048)
            dstart("bk", bk_sb[:], bk[:], 2048)
            for q in (1, 2, 3):
                for h in range(2):
                    dstart(f"x{q}{h}", xT_sb[:, q, 4 * h : 4 * h + 4], xT_r[:, q, 4 * h : 4 * h + 4], 524288)
            for cc in range(PCH):
                dstart(f"wp{cc}", wp_sb[:, cc, :], wp_r[:, cc, :], 262144)
            for q in (1, 2, 3):
                arrive[f"x{q}"] = max(arrive[f"x{q}0"], arrive[f"x{q}1"])
            arrive["x0"] = arrive["x03"]

            # setup ops on idle engines
            nc.vector.memset(tri_sb[:], 1.0)
            nc.gpsimd.affine_select(
                out=tri_sb[:],
                in_=tri_sb[:],
                compare_op=mybir.AluOpType.is_ge,
                fill=0.0,
                base=0,
                pattern=[[0, 2], [1, 128]],
                channel_multiplier=-1,
            )
            nc.vector.memset(V_sb[:, :, :, D], 1.0)
            nc.vector.memset(V_sb[:, :, :, D + 1 : 2 * D], 0.0)
            nc.gpsimd.memset(dum_sb[:], 0.5)
            nc.gpsimd.partition_broadcast(bv_sb[:], bv1_sb[:])

            # ---- virtual clocks / completion records ----
            clk = {"pe": 0.0, "sc": 0.0, "dve": 0.0, "pool": 0.0}
            qt_done = {}
            kt_done = {}
            v_done = {}
            at_done = {}

            # ---- PE warmup (dummy matmuls into the o banks) ----
            o0 = o_ps.tile([128, 512], f32, tag="o0")
            o1 = o_ps.tile([128, 512], f32, tag="o1")
            warm_cnt = [0]
            warm_ok = [True]

            def emit_warm():
                nc.tensor.matmul(
                    (o0 if warm_cnt[0] % 2 else o1)[:],
                    lhsT=dum_sb[:, 0:128],
                    rhs=dum_sb[:],
                    start=True,
                    stop=True,
                    skip_group_check=True,
                )
                warm_cnt[0] += 1

            for w in range(N_WARMUP):
                emit_warm()
            clk["pe"] = 600.0 + N_WARMUP * 330.0

            # ---- fill streams ----------------------------------------
            # A stream is a list of steps [ready, kind, emit, cost]; it holds
            # one f_ps slot from its first mm until its dve drain is emitted,
            # so drains are always emitted in-line when popped.

            def v_stream(tt):
                box = {}
                steps = []
                q, off = tt // 4, (tt % 4) * 128

                def mk_mm(cc):
                    def f():
                        if cc == 0:
                            box["ps"] = f_ps.tile([128, 512], f32, tag="f", name="fps")
                        nc.tensor.matmul(
                            box["ps"][:],
                            lhsT=xT_sb[:, q, cc, off : off + 128],
                            rhs=wv_sb[:, cc, :],
                            start=(cc == 0),
                            stop=(cc == CCH - 1),
                            skip_group_check=True,
                        )
                    return f

                for cc in range(CCH):
                    xk = f"x{q}{cc // 2}" if q == 0 else f"x{q}{cc // 4}"
                    rd = max(arrive.get(f"wv{cc // 2}", 0.0), arrive.get(xk, 0.0))
                    steps.append([rd, "mm", mk_mm(cc), MM])

                def drain():
                    if bias_zero and clk["sc"] + 900.0 < clk["pe"] - 700.0:
                        clk["sc"] = max(clk["sc"], clk["pe"]) + 900.0
                        nc.scalar.copy(
                            V_sb[:, tt, :, 0:D],
                            box["ps"][:].rearrange("p (h d) -> p h d", h=HL),
                        )
                        v_done[tt] = clk["sc"] + SEM
                    elif bias_zero:
                        clk["dve"] = max(clk["dve"], clk["pe"]) + DRAIN_C
                        nc.vector.tensor_copy(
                            V_sb[:, tt, :, 0:D],
                            box["ps"][:].rearrange("p (h d) -> p h d", h=HL),
                        )
                        v_done[tt] = clk["dve"] + SEM
                    else:
                        clk["dve"] = max(clk["dve"], clk["pe"]) + DRAIN_C
                        nc.vector.tensor_add(
                            V_sb[:, tt, :, 0:D],
                            box["ps"][:].rearrange("p (h d) -> p h d", h=HL),
                            bv_sb[:].rearrange("p (h d) -> p h d", h=HL),
                        )
                        v_done[tt] = clk["dve"] + SEM
                steps.append([0.0, "none", drain, 0.0])
                return steps

            def qk_stream(pair, which, q):
                box = {}
                steps = []
                w_sb, dst, b_sb, done = (
                    (wq_sb, QT_sb, bq_sb, qt_done),
                    (wk_sb, KT_sb, bk_sb, kt_done),
                )[which]

                def mk_mm(cc):
                    def f():
                        if cc == 0:
                            box["ps"] = f_ps.tile([128, 512], f32, tag="f", name="fps")
                        nc.tensor.matmul(
                            box["ps"][:],
                            lhsT=w_sb[:, pair, cc, :],
                            rhs=xT_sb[:, q, cc, :],
                            start=(cc == 0),
                            stop=(cc == CCH - 1),
                            skip_group_check=True,
                        )
                    return f

                wname = f"w{'qk'[which]}{pair}"
                for cc in range(CCH):
                    rd = max(arrive.get(wname, 0.0), arrive.get(f"x{q}", 0.0))
                    steps.append([rd, "mm", mk_mm(cc), MM])

                def drain():
                    if bias_zero and (q <= 1 or clk["sc"] + 800.0 < clk["pe"] - 700.0):
                        clk["sc"] = max(clk["sc"], clk["pe"]) + 800.0
                        nc.scalar.copy(
                            dst[:, pair, q * 512 : (q + 1) * 512], box["ps"][:]
                        )
                        done[(pair, q)] = clk["sc"] + SEM
                    elif bias_zero:
                        clk["dve"] = max(clk["dve"], clk["pe"]) + DRAIN_C
                        nc.vector.tensor_copy(
                            dst[:, pair, q * 512 : (q + 1) * 512], box["ps"][:]
                        )
                        done[(pair, q)] = clk["dve"] + SEM
                    else:
                        clk["dve"] = max(clk["dve"], clk["pe"]) + DRAIN_C
                        nc.vector.tensor_scalar_add(
                            dst[:, pair, q * 512 : (q + 1) * 512],
                            box["ps"][:],
                            b_sb[:, pair : pair + 1],
                        )
                        done[(pair, q)] = clk["dve"] + SEM
                steps.append([0.0, "none", drain, 0.0])
                return steps

            def proj_stream(tt, nh, so_box):
                ci = tt // 4
                box = {}
                steps = []

                def mk_mm(cc):
                    def f():
                        if cc == 0:
                            if nh == 0:
                                so_box["t"] = st_pool.tile(
                                    [128, 1024], bf, tag="so", name="so"
                                )
                            if attn_done[0]:
                                pool_, tag_ = tail_slots[tail_cnt[0] % len(tail_slots)]
                                tail_cnt[0] += 1
                            else:
                                pool_, tag_ = f_ps, "f"
                            box["ps"] = pool_.tile([128, 512], f32, tag=tag_, name="fps")
                        nc.tensor.matmul(
                            box["ps"][:],
                            lhsT=AT_sb[:, cc, tt * 128 : (tt + 1) * 128],
                            rhs=wp_sb[:, cc, nh * 512 : (nh + 1) * 512],
                            start=(cc == 0),
                            stop=(cc == PCH - 1),
                            skip_group_check=True,
                        )
                    return f

                for cc in range(PCH):
                    steps.append([("at", cc, ci), "mm", mk_mm(cc), MM])

                def drain_v():
                    so = so_box["t"]
                    nc.vector.tensor_copy(so[:, nh * 512 : (nh + 1) * 512], box["ps"][:])
                    if ci == NI - 1:
                        nc.sync.dma_start(
                            out=out[tt * 128 : (tt + 1) * 128, nh * 512 : (nh + 1) * 512],
                            in_=so[:, nh * 512 : (nh + 1) * 512],
                        )
                    elif nh == 1:
                        nc.sync.dma_start(out=out[tt * 128 : (tt + 1) * 128, :], in_=so[:])

                def drain_s():
                    so = so_box["t"]
                    nc.scalar.copy(so[:, nh * 512 : (nh + 1) * 512], box["ps"][:])
                    if ci == NI - 1:
                        nc.sync.dma_start(
                            out=out[tt * 128 : (tt + 1) * 128, nh * 512 : (nh + 1) * 512],
                            in_=so[:, nh * 512 : (nh + 1) * 512],
                        )
                    elif nh == 1:
                        nc.sync.dma_start(out=out[tt * 128 : (tt + 1) * 128, :], in_=so[:])

                def drain():
                    if attn_done[0] and tail_cnt[0] % 2:
                        clk["sc"] = max(clk["sc"], clk["pe"]) + 650.0
                        drain_s()
                    else:
                        clk["dve"] = max(clk["dve"], clk["pe"]) + DRAIN_C
                        drain_v()
                steps.append([0.0, "none", drain, 0.0])
                return steps

            streams = []
            key_of = {}

            def add(key, st):
                key_of[id(st)] = key
                streams.append(st)

            for tt in range(4):
                add(("v", tt), v_stream(tt))
            for p in range(NPAIR):
                add(("k", p, 0), qk_stream(p, 1, 0))
                add(("q", p, 0), qk_stream(p, 0, 0))
            for tt in range(4, 8):
                add(("v", tt), v_stream(tt))
            for p in range(NPAIR):
                add(("k", p, 1), qk_stream(p, 1, 1))
                add(("q", p, 1), qk_stream(p, 0, 1))
            for tt in range(8, 12):
                add(("v", tt), v_stream(tt))
            for p in range(NPAIR):
                add(("k", p, 2), qk_stream(p, 1, 2))
                add(("q", p, 2), qk_stream(p, 0, 2))
            for tt in range(12, 16):
                add(("v", tt), v_stream(tt))
            for p in range(NPAIR):
                add(("k", p, 3), qk_stream(p, 1, 3))
                add(("q", p, 3), qk_stream(p, 0, 3))
            for tt in range(TT):
                sb = {}
                add(("pj", tt, 0), proj_stream(tt, 0, sb))
                add(("pj", tt, 1), proj_stream(tt, 1, sb))

            active = []
            max_active = [2]
            attn_done = [False]
            tail_slots = [(f_ps, "f"), (ps_pool, "s"), (o_ps, "o0"), (f_ps, "f"), (ps_pool, "s"), (o_ps, "o1")]
            tail_cnt = [0]

            def refill_active():
                while len(active) < max_active[0] and streams:
                    active.append(streams.pop(0))

            def step_ready(st):
                r = st[0][0]
                if isinstance(r, tuple):
                    return at_done.get((r[1], r[2]), None)
                return r

            def exec_step(st):
                r, kind, emit, cost = st.pop(0)
                if isinstance(r, tuple):
                    r = at_done.get((r[1], r[2]), 0.0)
                if kind == "mm":
                    clk["pe"] = max(clk["pe"], r) + cost
                    emit()
                elif kind == "sc":
                    clk["sc"] = max(clk["sc"], clk["pe"]) + cost
                    emit()
                elif kind == "none":
                    emit()
                else:
                    clk["dve"] = max(clk["dve"], clk["pe"]) + cost
                    emit()
                if not st:
                    active.remove(st)
                    refill_active()

            def pump_fills(target, allow_drain=True):
                refill_active()
                while clk["pe"] + MM <= target:
                    pick = None
                    for st in active:
                        rd = step_ready(st)
                        if rd is None:
                            continue
                        if st[0][1] in ("dve", "sc", "none"):
                            if allow_drain:
                                pick = st
                                break
                            continue
                        if rd <= max(clk["pe"] + 120.0, target - MM):
                            pick = st
                            break
                    if pick is None:
                        if warm_ok[0] and warm_cnt[0] < 90 and active:
                            nxt = min(
                                (step_ready(st) for st in active if step_ready(st) is not None),
                                default=None,
                            )
                            if nxt is None:
                                return
                            while clk["pe"] + MM < min(nxt, target) and warm_cnt[0] < 90:
                                emit_warm()
                                clk["pe"] = clk["pe"] + MM
                            if clk["pe"] + MM > target:
                                return
                            continue
                        return
                    exec_step(pick)

            def ensure(pred):
                # force-run streams in order until pred() holds
                guard = 0
                while not pred() and guard < 10000:
                    guard += 1
                    refill_active()
                    picked = False
                    for st in active:
                        rd = step_ready(st)
                        if rd is not None:
                            exec_step(st)
                            picked = True
                            break
                    if not picked:
                        if active:
                            exec_step(active[0])
                        else:
                            break

            # ---- attention ----------------------------------------------
            slot_ring = []     # global exp-end ring (s pool bufs=2)
            o_free = [clk["pe"]]
            deferred = []      # deferred recip ops (DVE)

            def emit_chunk(pair, ci):
                njt = 4 * (ci + 1)
                ensure(lambda: (pair, ci) in qt_done)
                for q in range(ci + 1):
                    ensure(lambda q=q: (pair, q) in kt_done)
                ensure(lambda: njt - 1 in v_done)
                pt_tiles = {}

                def emit_S(jt):
                    rd = max(
                        qt_done.get((pair, ci), 0.0),
                        kt_done.get((pair, jt // 4), 0.0),
                        v_done.get(jt, 0.0),
                    )
                    if len(slot_ring) >= 2:
                        rd = max(rd, slot_ring[-2])
                    r = jt - 4 * ci
                    pump_fills(rd, allow_drain=(r < -1 or jt < 2))
                    st = ps_pool.tile([128, 2, 512], f32, tag="s")
                    i0 = max(0, 128 * r)
                    for s in range(2):
                        nc.tensor.matmul(
                            st[:, s, i0:512],
                            lhsT=KT_sb[64 * s : 64 * (s + 1), pair, jt * 128 : (jt + 1) * 128],
                            rhs=QT_sb[64 * s : 64 * (s + 1), pair, ci * 512 + i0 : (ci + 1) * 512],
                            start=True,
                            stop=True,
                            skip_group_check=True,
                        )
                    clk["pe"] = max(clk["pe"], rd) + 320.0
                    pt = p_pool.tile([128, 2, 512], bf, tag="p")
                    nc.scalar.activation(pt[:, :, i0:512], st[:, :, i0:512], Exp, scale=0.125)
                    ap = 2 * (512 - i0)
                    clk["sc"] = max(clk["sc"], clk["pe"] + SEM) + EXP_FIX + ap * EXP_EL
                    slot_ring.append(clk["sc"])
                    if r >= 0:
                        nc.vector.tensor_mul(
                            pt[:, :, i0 : i0 + 128], pt[:, :, i0 : i0 + 128], tri_sb[:]
                        )
                        clk["dve"] = max(clk["dve"], clk["sc"] + SEM) + MASK_C
                        pt_ready = clk["dve"] + SEM
                    else:
                        pt_ready = clk["sc"] + SEM
                    pt_tiles[jt] = (pt, i0, pt_ready)
                    if jt == 1 and deferred:
                        for f in deferred:
                            f()
                        deferred.clear()

                def emit_PV(jt):
                    warm_ok[0] = False
                    pt, i0, rdy = pt_tiles.pop(jt)
                    if jt == 0:
                        rdy = max(rdy, o_free[0])
                    pump_fills(rdy, allow_drain=(jt < 4 * ci - 1))
                    for s, ot in enumerate((o0, o1)):
                        nc.tensor.matmul(
                            ot[:, i0:512],
                            lhsT=V_sb[:, jt, 2 * pair + s, :],
                            rhs=pt[:, s, i0:512],
                            start=(jt == 0),
                            stop=(jt == njt - 1),
                            skip_group_check=True,
                        )
                    clk["pe"] = max(clk["pe"], rdy) + 2 * 240.0

                emit_S(0)
                for jt in range(1, njt):
                    emit_S(jt)
                    emit_PV(jt - 1)
                emit_PV(njt - 1)

                # ---- normalize ----
                oc0 = oc_pool.tile([128, 512], f32, tag="oc0")
                oc1 = oc_pool.tile([128, 512], f32, tag="oc1")
                dn0 = rb_pool.tile([1, 512], f32, tag="dn0")
                dn1 = rb_pool.tile([1, 512], f32, tag="dn1")
                nc.vector.tensor_copy(oc0[0:D, :], o0[0:D, :])
                nc.vector.tensor_copy(dn0[:], o0[D : D + 1, :])
                if (pair == NPAIR - 1 and ci == NI - 1) or clk["sc"] + 1400.0 < clk["pe"] - 900.0:
                    nc.scalar.copy(oc1[D : 2 * D, :], o1[0:D, :])
                    nc.scalar.copy(dn1[:], o1[D : D + 1, :])
                    clk["sc"] = max(clk["sc"], clk["pe"]) + 1300.0
                    clk["dve"] = max(clk["dve"], clk["pe"] + SEM) + OC_C + RECIP_C
                    o_free[0] = max(clk["dve"], clk["sc"])
                else:
                    nc.vector.tensor_copy(oc1[D : 2 * D, :], o1[0:D, :])
                    nc.vector.tensor_copy(dn1[:], o1[D : D + 1, :])
                    clk["dve"] = max(clk["dve"], clk["pe"] + SEM) + 2 * OC_C + 2 * RECIP_C
                    o_free[0] = clk["dve"]

                last = pair == NPAIR - 1 and ci == NI - 1

                def norm_tail(pair=pair, ci=ci, oc0=oc0, oc1=oc1, dn0=dn0, dn1=dn1, last=last):
                    rc = rb_pool.tile([1, 1024], f32, tag="rc")
                    rb = rb_pool.tile([128, 1024], f32, tag="rb")
                    if last:
                        # per-head pipeline: head0 mul overlaps head1 chain
                        nc.vector.reciprocal_approx_fast(rc[0:1, 0:512], dn0[:])
                        nc.gpsimd.partition_broadcast(rb[:, 0:512], rc[0:1, 0:512])
                        nc.vector.reciprocal_approx_fast(rc[0:1, 512:1024], dn1[:])
                        nc.gpsimd.partition_broadcast(rb[:, 512:1024], rc[0:1, 512:1024])
                        nc.vector.tensor_mul(
                            AT_sb[0:D, pair, ci * 512 : (ci + 1) * 512],
                            oc0[0:D, :],
                            rb[0:D, 0:512],
                        )
                        nc.vector.tensor_mul(
                            AT_sb[D : 2 * D, pair, ci * 512 : (ci + 1) * 512],
                            oc1[D : 2 * D, :],
                            rb[D : 2 * D, 512:1024],
                        )
                        clk["dve"] = max(clk["dve"], clk["pool"]) + 2 * RECIP_C + 2 * 900.0
                        at_done[(pair, ci)] = clk["dve"] + SEM
                        return
                    nc.vector.reciprocal_approx_fast(rc[0:1, 0:512], dn0[:])
                    nc.vector.reciprocal_approx_fast(rc[0:1, 512:1024], dn1[:])
                    clk["dve"] = clk["dve"] + 2 * RECIP_C
                    nc.gpsimd.partition_broadcast(rb[:], rc[:])
                    clk["pool"] = max(clk["pool"], clk["dve"] + SEM) + 2 * BCAST_C
                    nc.vector.tensor_mul(
                        AT_sb[0:D, pair, ci * 512 : (ci + 1) * 512],
                        oc0[0:D, :],
                        rb[0:D, 0:512],
                    )
                    nc.vector.tensor_mul(
                        AT_sb[D : 2 * D, pair, ci * 512 : (ci + 1) * 512],
                        oc1[D : 2 * D, :],
                        rb[D : 2 * D, 512:1024],
                    )
                    clk["dve"] = max(clk["dve"], clk["pool"] + SEM) + 2 * 900.0
                    at_done[(pair, ci)] = clk["dve"] + SEM

                deferred.append(norm_tail)

            for ci in range(NI):
                for pair in range(NPAIR):
                    emit_chunk(pair, ci)
            for f in deferred:
                f()
            deferred.clear()

            # ---- tail: drain all remaining fills ----
            attn_done[0] = True
            max_active[0] = 6
            refill_active()
            guard = 0
            while (active or streams) and guard < 20000:
                guard += 1
                refill_active()
                picked = False
                for st in active:
                    if step_ready(st) is not None:
                        exec_step(st)
                        picked = True
                        break
                if not picked and active:
                    exec_step(active[0])
    return nc


def _get_compiled(bias_zero):
    if bias_zero not in _compiled:
        from concourse import bacc

        nc = bacc.Bacc(
            "TRN2", target_bir_lowering=False, debug=False, num_devices=N_CORES
        )
        _build(nc, bias_zero)
        nc.compile()
        _compiled[bias_zero] = nc
    return _compiled[bias_zero]


def _shard_inputs(x, w_qkv, b_qkv, w_proj):
    """Build the 8 per-core input dicts (host-side transpose/slice/cast)."""
    in_maps = []
    wq_f, wk_f, wv_f = w_qkv[:, :C], w_qkv[:, C : 2 * C], w_qkv[:, 2 * C :]
    for c in range(N_CORES):
        b, g = c // 2, c % 2
        sl = slice(g * CL, (g + 1) * CL)
        bqs = np.ascontiguousarray(b_qkv[0 * C :][sl].reshape(NPAIR, 128).T)
        bks = np.ascontiguousarray(b_qkv[1 * C :][sl].reshape(NPAIR, 128).T)
        bvs = np.ascontiguousarray(b_qkv[2 * C :][sl][None, :])
        wq_p = np.ascontiguousarray(
            wq_f[:, sl].reshape(CCH, 128, NPAIR, 128).transpose(2, 1, 0, 3)
            .reshape(NPAIR, 128, CCH * 128)
        )
        wk_p = np.ascontiguousarray(
            wk_f[:, sl].reshape(CCH, 128, NPAIR, 128).transpose(2, 1, 0, 3)
            .reshape(NPAIR, 128, CCH * 128)
        )
        xT_p = np.ascontiguousarray(
            x[b].T.reshape(CCH, 128, 4, 512).transpose(1, 2, 0, 3)
            .reshape(128, 4 * CCH * 512)
        )
        wv_p = np.ascontiguousarray(
            wv_f[:, sl].reshape(CCH, 128, CL).transpose(1, 0, 2)
            .reshape(128, CCH * CL)
        )
        in_maps.append(
            {
                "xT": xT_p.astype(BF16),
                "wq": wq_p.astype(BF16),
                "wk": wk_p.astype(BF16),
                "wv": wv_p.astype(BF16),
                "bq": bqs.astype(np.float32),
                "bk": bks.astype(np.float32),
                "bv": bvs.astype(np.float32),
                "wp": np.ascontiguousarray(w_proj[sl, :]).astype(BF16),
            }
        )
    return in_maps


def kernel(x, w_qkv, b_qkv, w_proj, b_proj, _trace=False, _tmpdir=None):
    from concourse.bass_utils import run_bass_kernel_spmd

    x = np.asarray(x, dtype=np.float32)
    w_qkv = np.asarray(w_qkv, dtype=np.float32)
    b_qkv = np.asarray(b_qkv, dtype=np.float32)
    w_proj = np.asarray(w_proj, dtype=np.float32)
    b_proj = np.asarray(b_proj, dtype=np.float32)

    nc = _get_compiled(not b_qkv.any())
    in_maps = _shard_inputs(x, w_qkv, b_qkv, w_proj)
    res = run_bass_kernel_spmd(
        nc,
        in_maps,
        core_ids=list(range(N_CORES)),
        trace=_trace,
        tmpdir=_tmpdir,
    )
    out = np.empty((B, T, C), dtype=np.float32)
    for b in range(B):
        out[b] = (
            res.results[2 * b]["out"].astype(np.float32)
            + res.results[2 * b + 1]["out"].astype(np.float32)
            + b_proj
        )
    kernel._last_result = res
    return out


# revision 29
# speedup vs baseline: 1.0227x; 1.0227x over previous
"""Causal self-attention on 8 Trainium2 NeuronCores.

Problem (hardcoded): B=4, T=2048, C=1024, H=16, D=64.
  qkv = x @ w_qkv + b_qkv ; per-head causal softmax attention ; out = attn @ w_proj + b_proj

Sharding (per hint): tensor-parallel over heads x data-parallel over batch.
  core c -> batch b = c // 2, head group g = c % 2 (heads g*8 .. g*8+7).
Each core computes QKV for its 8 heads, causal attention, and a partial
projection (its 512 input channels of w_proj). Host sums the two partials per
batch and adds b_proj.

v2: virtual-clock emitter. Attention (S -> exp -> PV) is paced by the Scalar
engine's exp; all other PE work (QKV projections, output projection) is
emitted matmul-granular into the predicted exp-stall windows from a dedicated
2-bank PSUM fill pool, so fills never wait on the exp-paced S ring (which
shrinks to 2 slots). Vector load is cut by merging the softmax-denominator
row into the O-tile drain (one copy instead of two), reciprocal in place, and
moving the normalize broadcast+multiply to the (otherwise idle) GpSimd/Pool
engine. PE warmup matmuls run at t=0 against a memset dummy tile so the
p-state ramp happens before real work; first DMA parcels are split so the
initial transfers fan across multiple HW queues.
"""

import numpy as np
import ml_dtypes

B, T, C, H, D = 4, 2048, 1024, 16, 64
HL = H // 2          # heads per core
CL = HL * D          # local channels (512)
NPAIR = HL // 2      # head pairs per core (4)
CCH = C // 128       # contraction chunks for qkv (8)
PCH = CL // 128      # contraction chunks for proj (4)
TT = T // 128        # t tiles (16)
NI = T // 512        # i chunks (4)
N_CORES = 8
BF16 = ml_dtypes.bfloat16

# ---- virtual-clock cost constants (ns) ----
MM = 217.0        # N=512 matmul issue slot
SEM = 150.0       # cross-engine semaphore hop
EXP_FIX = 190.0   # activation fixed overhead
EXP_EL = 1.0 / 1.2
MASK_C = 330.0
OC_C = 700.0
RECIP_C = 560.0
DRAIN_C = 800.0   # [128,512] psum->sbuf drain on DVE
BCAST_C = 650.0
PMUL_C = 1150.0
DMA_BW = 0.0029   # ns per byte (~345 GB/s aggregate)

N_WARMUP = 24     # dummy PE warmup matmuls

_compiled = {}


def _build(nc, bias_zero=False):
    import concourse.tile as tile
    from concourse import mybir

    bf = mybir.dt.bfloat16
    f32 = mybir.dt.float32
    Exp = mybir.ActivationFunctionType.Exp
    Ident = mybir.ActivationFunctionType.Identity

    xT = nc.dram_tensor("xT", [128, 4 * CCH * 512], bf, kind="ExternalInput").ap()
    wq = nc.dram_tensor("wq", [NPAIR, 128, CCH * 128], bf, kind="ExternalInput").ap()
    wk = nc.dram_tensor("wk", [NPAIR, 128, CCH * 128], bf, kind="ExternalInput").ap()
    wv = nc.dram_tensor("wv", [128, CCH * CL], bf, kind="ExternalInput").ap()
    bq = nc.dram_tensor("bq", [128, NPAIR], f32, kind="ExternalInput").ap()
    bk = nc.dram_tensor("bk", [128, NPAIR], f32, kind="ExternalInput").ap()
    bv = nc.dram_tensor("bv", [1, CL], f32, kind="ExternalInput").ap()
    wp = nc.dram_tensor("wp", [CL, C], bf, kind="ExternalInput").ap()
    out = nc.dram_tensor("out", [T, C], bf, kind="ExternalOutput").ap()

    xT_r = xT.rearrange("p (q cc t) -> p q cc t", q=4, cc=CCH)
    wv_r = wv.rearrange("p (cc m) -> p cc m", cc=CCH)
    wp_r = wp.rearrange("(cc p) n -> p cc n", p=128)
    wq_r = wq.rearrange("a p (cc m) -> a p cc m", m=128)
    wk_r = wk.rearrange("a p (cc m) -> a p cc m", m=128)

    with tile.TileContext(nc) as tc:
        import contextlib

        with contextlib.ExitStack() as ctx:
            persist = ctx.enter_context(tc.tile_pool(name="persist", bufs=1))
            # PSUM: s-ring 2x[128,2,512] (4 banks) + o0/o1 (2 banks) +
            # fill pool 2x[128,512] (2 banks) = 8 banks.
            ps_pool = ctx.enter_context(tc.tile_pool(name="ps_pool", bufs=2, space="PSUM"))
            o_ps = ctx.enter_context(tc.tile_pool(name="o_ps", bufs=1, space="PSUM"))
            f_ps = ctx.enter_context(tc.tile_pool(name="f_ps", bufs=2, space="PSUM"))
            p_pool = ctx.enter_context(tc.tile_pool(name="p_pool", bufs=8))
            oc_pool = ctx.enter_context(tc.tile_pool(name="oc_pool", bufs=2))
            rb_pool = ctx.enter_context(tc.tile_pool(name="rb_pool", bufs=2))
            st_pool = ctx.enter_context(tc.tile_pool(name="st_pool", bufs=4))

            # ---- persistent SBUF tensors ----
            xT_sb = persist.tile([128, 4, CCH, 512], bf)
            wq_sb = persist.tile([128, NPAIR, CCH, 128], bf)
            wk_sb = persist.tile([128, NPAIR, CCH, 128], bf)
            wv_sb = persist.tile([128, CCH, CL], bf)
            wp_sb = persist.tile([128, PCH, C], bf)
            bq_sb = persist.tile([128, NPAIR], f32)
            bk_sb = persist.tile([128, NPAIR], f32)
            bv1_sb = persist.tile([1, CL], f32)
            bv_sb = persist.tile([128, CL], f32)
            QT_sb = persist.tile([128, NPAIR, T], bf)
            KT_sb = persist.tile([128, NPAIR, T], bf)
            V_sb = persist.tile([128, TT, HL, 2 * D], bf)
            AT_sb = persist.tile([128, PCH, T], bf)
            tri_sb = persist.tile([128, 2, 128], bf)
            dum_sb = persist.tile([128, 512], bf)

            # ---- DMA emission (sync queue), first parcels split fine ----
            dma_t = [3500.0]
            arrive = {}

            def dstart(name, out_ap, in_ap, nbytes):
                nc.sync.dma_start(out=out_ap, in_=in_ap)
                dma_t[0] = dma_t[0] + 565.0
                t = max(dma_t[0] + 1500.0, arrive.get("_last", 0.0)) + nbytes * DMA_BW
                arrive[name] = t + 1200.0
                arrive["_last"] = t

            dstart("bv", bv1_sb[:], bv[:], 1024)
            # first wave: fan the first four big parcels across four idle
            # engine queues so their HWDGE issues run concurrently
            nc.scalar.dma_start(out=wv_sb[:, 0:2], in_=wv_r[:, 0:2])
            nc.gpsimd.dma_start(out=xT_sb[:, 0, 0:2], in_=xT_r[:, 0, 0:2])
            nc.gpsimd.dma_start(out=wv_sb[:, 2:4], in_=wv_r[:, 2:4])
            arrive["wv0"] = 5600.0
            arrive["x00"] = 5900.0
            arrive["wv1"] = 6400.0
            for h in range(1, 4):
                if h > 1:
                    dstart(f"wv{h}", wv_sb[:, 2 * h : 2 * h + 2], wv_r[:, 2 * h : 2 * h + 2], 262144)
                dstart(f"x0{h}", xT_sb[:, 0, 2 * h : 2 * h + 2], xT_r[:, 0, 2 * h : 2 * h + 2], 262144)
            for p in range(NPAIR):
                dstart(f"wk{p}", wk_sb[:, p], wk_r[p], 262144)
                dstart(f"wq{p}", wq_sb[:, p], wq_r[p], 262144)
            dstart("bq", bq_sb[:], bq[:], 2048)
            dstart("bk", bk_sb[:], bk[:], 2048)
            for q in (1, 2, 3):
                for h in range(2):
                    dstart(f"x{q}{h}", xT_sb[:, q, 4 * h : 4 * h + 4], xT_r[:, q, 4 * h : 4 * h + 4], 524288)
            for cc in range(PCH):
                dstart(f"wp{cc}", wp_sb[:, cc, :], wp_r[:, cc, :], 262144)
            for q in (1, 2, 3):
                arrive[f"x{q}"] = max(arrive[f"x{q}0"], arrive[f"x{q}1"])
            arrive["x0"] = arrive["x03"]

            # setup ops on idle engines
            nc.vector.memset(tri_sb[:], 1.0)
            nc.gpsimd.affine_select(
                out=tri_sb[:],
                in_=tri_sb[:],
                compare_op=mybir.AluOpType.is_ge,
                fill=0.0,
                base=0,
                pattern=[[0, 2], [1, 128]],
                channel_multiplier=-1,
            )
            nc.vector.memset(V_sb[:, :, :, D], 1.0)
            nc.vector.memset(V_sb[:, :, :, D + 1 : 2 * D], 0.0)
            nc.gpsimd.memset(dum_sb[:], 0.5)
            nc.gpsimd.partition_broadcast(bv_sb[:], bv1_sb[:])

            # ---- virtual clocks / completion records ----
            clk = {"pe": 0.0, "sc": 0.0, "dve": 0.0, "pool": 0.0}
            qt_done = {}
            kt_done = {}
            v_done = {}
            at_done = {}

            # ---- PE warmup (dummy matmuls into the o banks) ----
            o0 = o_ps.tile([128, 512], f32, tag="o0")
            o1 = o_ps.tile([128, 512], f32, tag="o1")
            warm_cnt = [0]
            warm_ok = [True]

            def emit_warm():
                nc.tensor.matmul(
                    (o0 if warm_cnt[0] % 2 else o1)[:],
                    lhsT=dum_sb[:, 0:128],
                    rhs=dum_sb[:],
                    start=True,
                    stop=True,
                    skip_group_check=True,
                )
                warm_cnt[0] += 1

            for w in range(N_WARMUP):
                emit_warm()
            clk["pe"] = 600.0 + N_WARMUP * 330.0

            # ---- fill streams ----------------------------------------
            # A stream is a list of steps [ready, kind, emit, cost]; it holds
            # one f_ps slot from its first mm until its dve drain is emitted,
            # so drains are always emitted in-line when popped.

            def v_stream(tt):
                box = {}
                steps = []
                q, off = tt // 4, (tt % 4) * 128

                def mk_mm(cc):
                    def f():
                        if cc == 0:
                            box["ps"] = f_ps.tile([128, 512], f32, tag="f", name="fps")
                        nc.tensor.matmul(
                            box["ps"][:],
                            lhsT=xT_sb[:, q, cc, off : off + 128],
                            rhs=wv_sb[:, cc, :],
                            start=(cc == 0),
                            stop=(cc == CCH - 1),
                            skip_group_check=True,
                        )
                    return f

                for cc in range(CCH):
                    xk = f"x{q}{cc // 2}" if q == 0 else f"x{q}{cc // 4}"
                    rd = max(arrive.get(f"wv{cc // 2}", 0.0), arrive.get(xk, 0.0))
                    steps.append([rd, "mm", mk_mm(cc), MM])

                def drain():
                    if bias_zero and clk["sc"] + 900.0 < clk["pe"] - 700.0:
                        clk["sc"] = max(clk["sc"], clk["pe"]) + 900.0
                        nc.scalar.copy(
                            V_sb[:, tt, :, 0:D],
                            box["ps"][:].rearrange("p (h d) -> p h d", h=HL),
                        )
                        v_done[tt] = clk["sc"] + SEM
                    elif bias_zero:
                        clk["dve"] = max(clk["dve"], clk["pe"]) + DRAIN_C
                        nc.vector.tensor_copy(
                            V_sb[:, tt, :, 0:D],
                            box["ps"][:].rearrange("p (h d) -> p h d", h=HL),
                        )
                        v_done[tt] = clk["dve"] + SEM
                    else:
                        clk["dve"] = max(clk["dve"], clk["pe"]) + DRAIN_C
                        nc.vector.tensor_add(
                            V_sb[:, tt, :, 0:D],
                            box["ps"][:].rearrange("p (h d) -> p h d", h=HL),
                            bv_sb[:].rearrange("p (h d) -> p h d", h=HL),
                        )
                        v_done[tt] = clk["dve"] + SEM
                steps.append([0.0, "none", drain, 0.0])
                return steps

            def qk_stream(pair, which, q):
                box = {}
                steps = []
                w_sb, dst, b_sb, done = (
                    (wq_sb, QT_sb, bq_sb, qt_done),
                    (wk_sb, KT_sb, bk_sb, kt_done),
                )[which]

                def mk_mm(cc):
                    def f():
                        if cc == 0:
                            box["ps"] = f_ps.tile([128, 512], f32, tag="f", name="fps")
                        nc.tensor.matmul(
                            box["ps"][:],
                            lhsT=w_sb[:, pair, cc, :],
                            rhs=xT_sb[:, q, cc, :],
                            start=(cc == 0),
                            stop=(cc == CCH - 1),
                            skip_group_check=True,
                        )
                    return f

                wname = f"w{'qk'[which]}{pair}"
                for cc in range(CCH):
                    rd = max(arrive.get(wname, 0.0), arrive.get(f"x{q}", 0.0))
                    steps.append([rd, "mm", mk_mm(cc), MM])

                def drain():
                    if bias_zero and clk["sc"] + 800.0 < clk["pe"] - 700.0:
                        clk["sc"] = max(clk["sc"], clk["pe"]) + 800.0
                        nc.scalar.copy(
                            dst[:, pair, q * 512 : (q + 1) * 512], box["ps"][:]
                        )
                        done[(pair, q)] = clk["sc"] + SEM
                    elif bias_zero:
                        clk["dve"] = max(clk["dve"], clk["pe"]) + DRAIN_C
                        nc.vector.tensor_copy(
                            dst[:, pair, q * 512 : (q + 1) * 512], box["ps"][:]
                        )
                        done[(pair, q)] = clk["dve"] + SEM
                    else:
                        clk["dve"] = max(clk["dve"], clk["pe"]) + DRAIN_C
                        nc.vector.tensor_scalar_add(
                            dst[:, pair, q * 512 : (q + 1) * 512],
                            box["ps"][:],
                            b_sb[:, pair : pair + 1],
                        )
                        done[(pair, q)] = clk["dve"] + SEM
                steps.append([0.0, "none", drain, 0.0])
                return steps

            def proj_stream(tt, nh, so_box):
                ci = tt // 4
                box = {}
                steps = []

                def mk_mm(cc):
                    def f():
                        if cc == 0:
                            if nh == 0:
                                so_box["t"] = st_pool.tile(
                                    [128, 1024], bf, tag="so", name="so"
                                )
                            if attn_done[0]:
                                pool_, tag_ = tail_slots[tail_cnt[0] % len(tail_slots)]
                                tail_cnt[0] += 1
                            else:
                                pool_, tag_ = f_ps, "f"
                            box["ps"] = pool_.tile([128, 512], f32, tag=tag_, name="fps")
                        nc.tensor.matmul(
                            box["ps"][:],
                            lhsT=AT_sb[:, cc, tt * 128 : (tt + 1) * 128],
                            rhs=wp_sb[:, cc, nh * 512 : (nh + 1) * 512],
                            start=(cc == 0),
                            stop=(cc == PCH - 1),
                            skip_group_check=True,
                        )
                    return f

                for cc in range(PCH):
                    steps.append([("at", cc, ci), "mm", mk_mm(cc), MM])

                def drain_v():
                    so = so_box["t"]
                    nc.vector.tensor_copy(so[:, nh * 512 : (nh + 1) * 512], box["ps"][:])
                    if ci == NI - 1:
                        nc.sync.dma_start(
                            out=out[tt * 128 : (tt + 1) * 128, nh * 512 : (nh + 1) * 512],
                            in_=so[:, nh * 512 : (nh + 1) * 512],
                        )
                    elif nh == 1:
                        nc.sync.dma_start(out=out[tt * 128 : (tt + 1) * 128, :], in_=so[:])

                def drain_s():
                    so = so_box["t"]
                    nc.scalar.copy(so[:, nh * 512 : (nh + 1) * 512], box["ps"][:])
                    if ci == NI - 1:
                        nc.sync.dma_start(
                            out=out[tt * 128 : (tt + 1) * 128, nh * 512 : (nh + 1) * 512],
                            in_=so[:, nh * 512 : (nh + 1) * 512],
                        )
                    elif nh == 1:
                        nc.sync.dma_start(out=out[tt * 128 : (tt + 1) * 128, :], in_=so[:])

                def drain():
                    if attn_done[0] and tail_cnt[0] % 2:
                        clk["sc"] = max(clk["sc"], clk["pe"]) + 650.0
                        drain_s()
                    else:
                        clk["dve"] = max(clk["dve"], clk["pe"]) + DRAIN_C
                        drain_v()
                steps.append([0.0, "none", drain, 0.0])
                return steps

            streams = []
            key_of = {}

            def add(key, st):
                key_of[id(st)] = key
                streams.append(st)

            for tt in range(4):
                add(("v", tt), v_stream(tt))
            for p in range(NPAIR):
                add(("k", p, 0), qk_stream(p, 1, 0))
                add(("q", p, 0), qk_stream(p, 0, 0))
            for tt in range(4, 8):
                add(("v", tt), v_stream(tt))
            for p in range(NPAIR):
                add(("k", p, 1), qk_stream(p, 1, 1))
                add(("q", p, 1), qk_stream(p, 0, 1))
            for tt in range(8, 12):
                add(("v", tt), v_stream(tt))
            for p in range(NPAIR):
                add(("k", p, 2), qk_stream(p, 1, 2))
                add(("q", p, 2), qk_stream(p, 0, 2))
            for tt in range(12, 16):
                add(("v", tt), v_stream(tt))
            for p in range(NPAIR):
                add(("k", p, 3), qk_stream(p, 1, 3))
                add(("q", p, 3), qk_stream(p, 0, 3))
            for tt in range(TT):
                sb = {}
                add(("pj", tt, 0), proj_stream(tt, 0, sb))
                add(("pj", tt, 1), proj_stream(tt, 1, sb))

            active = []
            max_active = [2]
            attn_done = [False]
            tail_slots = [(f_ps, "f"), (ps_pool, "s"), (o_ps, "o0"), (f_ps, "f"), (ps_pool, "s"), (o_ps, "o1")]
            tail_cnt = [0]

            def refill_active():
                while len(active) < max_active[0] and streams:
                    active.append(streams.pop(0))

            def step_ready(st):
                r = st[0][0]
                if isinstance(r, tuple):
                    return at_done.get((r[1], r[2]), None)
                return r

            def exec_step(st):
                r, kind, emit, cost = st.pop(0)
                if isinstance(r, tuple):
                    r = at_done.get((r[1], r[2]), 0.0)
                if kind == "mm":
                    clk["pe"] = max(clk["pe"], r) + cost
                    emit()
                elif kind == "sc":
                    clk["sc"] = max(clk["sc"], clk["pe"]) + cost
                    emit()
                elif kind == "none":
                    emit()
                else:
                    clk["dve"] = max(clk["dve"], clk["pe"]) + cost
                    emit()
                if not st:
                    active.remove(st)
                    refill_active()

            def pump_fills(target, allow_drain=True):
                refill_active()
                while clk["pe"] + MM <= target:
                    pick = None
                    for st in active:
                        rd = step_ready(st)
                        if rd is None:
                            continue
                        if st[0][1] in ("dve", "sc", "none"):
                            if allow_drain:
                                pick = st
                                break
                            continue
                        if rd <= max(clk["pe"] + 120.0, target - MM):
                            pick = st
                            break
                    if pick is None:
                        if warm_ok[0] and warm_cnt[0] < 90 and active:
                            nxt = min(
                                (step_ready(st) for st in active if step_ready(st) is not None),
                                default=None,
                            )
                            if nxt is None:
                                return
                            while clk["pe"] + MM < min(nxt, target) and warm_cnt[0] < 90:
                                emit_warm()
                                clk["pe"] = clk["pe"] + MM
                            if clk["pe"] + MM > target:
                                return
                            continue
                        return
                    exec_step(pick)

            def ensure(pred):
                # force-run streams in order until pred() holds
                guard = 0
                while not pred() and guard < 10000:
                    guard += 1
                    refill_active()
                    picked = False
                    for st in active:
                        rd = step_ready(st)
                        if rd is not None:
                            exec_step(st)
                            picked = True
                            break
                    if not picked:
                        if active:
                            exec_step(active[0])
                        else:
                            break

            # ---- attention ----------------------------------------------
            slot_ring = []     # global exp-end ring (s pool bufs=2)
            o_free = [clk["pe"]]
            deferred = []      # deferred recip ops (DVE)

            def emit_chunk(pair, ci):
                njt = 4 * (ci + 1)
                ensure(lambda: (pair, ci) in qt_done)
                for q in range(ci + 1):
                    ensure(lambda q=q: (pair, q) in kt_done)
                ensure(lambda: njt - 1 in v_done)
                pt_tiles = {}

                def emit_S(jt):
                    rd = max(
                        qt_done.get((pair, ci), 0.0),
                        kt_done.get((pair, jt // 4), 0.0),
                        v_done.get(jt, 0.0),
                    )
                    if len(slot_ring) >= 2:
                        rd = max(rd, slot_ring[-2])
                    r = jt - 4 * ci
                    pump_fills(rd, allow_drain=(r < -1 or jt < 2))
                    st = ps_pool.tile([128, 2, 512], f32, tag="s")
                    i0 = max(0, 128 * r)
                    for s in range(2):
                        nc.tensor.matmul(
                            st[:, s, i0:512],
                            lhsT=KT_sb[64 * s : 64 * (s + 1), pair, jt * 128 : (jt + 1) * 128],
                            rhs=QT_sb[64 * s : 64 * (s + 1), pair, ci * 512 + i0 : (ci + 1) * 512],
                            start=True,
                            stop=True,
                            skip_group_check=True,
                        )
                    clk["pe"] = max(clk["pe"], rd) + 320.0
                    pt = p_pool.tile([128, 2, 512], bf, tag="p")
                    nc.scalar.activation(pt[:, :, i0:512], st[:, :, i0:512], Exp, scale=0.125)
                    ap = 2 * (512 - i0)
                    clk["sc"] = max(clk["sc"], clk["pe"] + SEM) + EXP_FIX + ap * EXP_EL
                    slot_ring.append(clk["sc"])
                    if r >= 0:
                        nc.vector.tensor_mul(
                            pt[:, :, i0 : i0 + 128], pt[:, :, i0 : i0 + 128], tri_sb[:]
                        )
                        clk["dve"] = max(clk["dve"], clk["sc"] + SEM) + MASK_C
                        pt_ready = clk["dve"] + SEM
                    else:
                        pt_ready = clk["sc"] + SEM
                    pt_tiles[jt] = (pt, i0, pt_ready)
                    if jt == 1 and deferred:
                        for f in deferred:
                            f()
                        deferred.clear()

                def emit_PV(jt):
                    warm_ok[0] = False
                    pt, i0, rdy = pt_tiles.pop(jt)
                    if jt == 0:
                        rdy = max(rdy, o_free[0])
                    pump_fills(rdy, allow_drain=(jt < 4 * ci - 1))
                    for s, ot in enumerate((o0, o1)):
                        nc.tensor.matmul(
                            ot[:, i0:512],
                            lhsT=V_sb[:, jt, 2 * pair + s, :],
                            rhs=pt[:, s, i0:512],
                            start=(jt == 0),
                            stop=(jt == njt - 1),
                            skip_group_check=True,
                        )
                    clk["pe"] = max(clk["pe"], rdy) + 2 * 240.0

                emit_S(0)
                for jt in range(1, njt):
                    emit_S(jt)
                    emit_PV(jt - 1)
                emit_PV(njt - 1)

                # ---- normalize ----
                oc0 = oc_pool.tile([128, 512], f32, tag="oc0")
                oc1 = oc_pool.tile([128, 512], f32, tag="oc1")
                dn0 = rb_pool.tile([1, 512], f32, tag="dn0")
                dn1 = rb_pool.tile([1, 512], f32, tag="dn1")
                nc.vector.tensor_copy(oc0[0:D, :], o0[0:D, :])
                nc.vector.tensor_copy(dn0[:], o0[D : D + 1, :])
                if (pair == NPAIR - 1 and ci == NI - 1) or clk["sc"] + 1400.0 < clk["pe"] - 900.0:
                    nc.scalar.copy(oc1[D : 2 * D, :], o1[0:D, :])
                    nc.scalar.copy(dn1[:], o1[D : D + 1, :])
                    clk["sc"] = max(clk["sc"], clk["pe"]) + 1300.0
                    clk["dve"] = max(clk["dve"], clk["pe"] + SEM) + OC_C + RECIP_C
                    o_free[0] = max(clk["dve"], clk["sc"])
                else:
                    nc.vector.tensor_copy(oc1[D : 2 * D, :], o1[0:D, :])
                    nc.vector.tensor_copy(dn1[:], o1[D : D + 1, :])
                    clk["dve"] = max(clk["dve"], clk["pe"] + SEM) + 2 * OC_C + 2 * RECIP_C
                    o_free[0] = clk["dve"]

                last = pair == NPAIR - 1 and ci == NI - 1

                def norm_tail(pair=pair, ci=ci, oc0=oc0, oc1=oc1, dn0=dn0, dn1=dn1, last=last):
                    rc = rb_pool.tile([1, 1024], f32, tag="rc")
                    rb = rb_pool.tile([128, 1024], f32, tag="rb")
                    if last:
                        # per-head pipeline: head0 mul overlaps head1 chain
                        nc.vector.reciprocal_approx_fast(rc[0:1, 0:512], dn0[:])
                        nc.gpsimd.partition_broadcast(rb[:, 0:512], rc[0:1, 0:512])
                        nc.vector.reciprocal_approx_fast(rc[0:1, 512:1024], dn1[:])
                        nc.gpsimd.partition_broadcast(rb[:, 512:1024], rc[0:1, 512:1024])
                        nc.vector.tensor_mul(
                            AT_sb[0:D, pair, ci * 512 : (ci + 1) * 512],
                            oc0[0:D, :],
                            rb[0:D, 0:512],
                        )
                        nc.vector.tensor_mul(
                            AT_sb[D : 2 * D, pair, ci * 512 : (ci + 1) * 512],
                            oc1[D : 2 * D, :],
                            rb[D : 2 * D, 512:1024],
                        )
                        clk["dve"] = max(clk["dve"], clk["pool"]) + 2 * RECIP_C + 2 * 900.0
                        at_done[(pair, ci)] = clk["dve"] + SEM
                        return
                    nc.vector.reciprocal_approx_fast(rc[0:1, 0:512], dn0[:])
                    nc.vector.reciprocal_approx_fast(rc[0:1, 512:1024], dn1[:])
                    clk["dve"] = clk["dve"] + 2 * RECIP_C
                    nc.gpsimd.partition_broadcast(rb[:], rc[:])
                    clk["pool"] = max(clk["pool"], clk["dve"] + SEM) + 2 * BCAST_C
                    nc.vector.tensor_mul(
                        AT_sb[0:D, pair, ci * 512 : (ci + 1) * 512],
                        oc0[0:D, :],
                        rb[0:D, 0:512],
                    )
                    nc.vector.tensor_mul(
                        AT_sb[D : 2 * D, pair, ci * 512 : (ci + 1) * 512],
                        oc1[D : 2 * D, :],
                        rb[D : 2 * D, 512:1024],
                    )
                    clk["dve"] = max(clk["dve"], clk["pool"] + SEM) + 2 * 900.0
                    at_done[(pair, ci)] = clk["dve"] + SEM

                deferred.append(norm_tail)

            for ci in range(NI):
                for pair in range(NPAIR):
                    emit_chunk(pair, ci)
            for f in deferred:
                f()
            deferred.clear()

            # ---- tail: drain all remaining fills ----
            attn_done[0] = True
            max_active[0] = 6
            refill_active()
            guard = 0
            while (active or streams) and guard < 20000:
                guard += 1
                refill_active()
                picked = False
                for st in active:
                    if step_ready(st) is not None:
                        exec_step(st)
                        picked = True
                        break
                if not picked and active:
                    exec_step(active[0])
    return nc


def _get_compiled(bias_zero):
    if bias_zero not in _compiled:
        from concourse import bacc

        nc = bacc.Bacc(
            "TRN2", target_bir_lowering=False, debug=False, num_devices=N_CORES
        )
        _build(nc, bias_zero)
        nc.compile()
        _compiled[bias_zero] = nc
    return _compiled[bias_zero]


def _shard_inputs(x, w_qkv, b_qkv, w_proj):
    """Build the 8 per-core input dicts (host-side transpose/slice/cast)."""
    in_maps = []
    wq_f, wk_f, wv_f = w_qkv[:, :C], w_qkv[:, C : 2 * C], w_qkv[:, 2 * C :]
    for c in range(N_CORES):
        b, g = c // 2, c % 2
        sl = slice(g * CL, (g + 1) * CL)
        bqs = np.ascontiguousarray(b_qkv[0 * C :][sl].reshape(NPAIR, 128).T)
        bks = np.ascontiguousarray(b_qkv[1 * C :][sl].reshape(NPAIR, 128).T)
        bvs = np.ascontiguousarray(b_qkv[2 * C :][sl][None, :])
        wq_p = np.ascontiguousarray(
            wq_f[:, sl].reshape(CCH, 128, NPAIR, 128).transpose(2, 1, 0, 3)
            .reshape(NPAIR, 128, CCH * 128)
        )
        wk_p = np.ascontiguousarray(
            wk_f[:, sl].reshape(CCH, 128, NPAIR, 128).transpose(2, 1, 0, 3)
            .reshape(NPAIR, 128, CCH * 128)
        )
        xT_p = np.ascontiguousarray(
            x[b].T.reshape(CCH, 128, 4, 512).transpose(1, 2, 0, 3)
            .reshape(128, 4 * CCH * 512)
        )
        wv_p = np.ascontiguousarray(
            wv_f[:, sl].reshape(CCH, 128, CL).transpose(1, 0, 2)
            .reshape(128, CCH * CL)
        )
        in_maps.append(
            {
                "xT": xT_p.astype(BF16),
                "wq": wq_p.astype(BF16),
                "wk": wk_p.astype(BF16),
                "wv": wv_p.astype(BF16),
                "bq": bqs.astype(np.float32),
                "bk": bks.astype(np.float32),
                "bv": bvs.astype(np.float32),
                "wp": np.ascontiguousarray(w_proj[sl, :]).astype(BF16),
            }
        )
    return in_maps


def kernel(x, w_qkv, b_qkv, w_proj, b_proj, _trace=False, _tmpdir=None):
    from concourse.bass_utils import run_bass_kernel_spmd

    x = np.asarray(x, dtype=np.float32)
    w_qkv = np.asarray(w_qkv, dtype=np.float32)
    b_qkv = np.asarray(b_qkv, dtype=np.float32)
    w_proj = np.asarray(w_proj, dtype=np.float32)
    b_proj = np.asarray(b_proj, dtype=np.float32)

    nc = _get_compiled(not b_qkv.any())
    in_maps = _shard_inputs(x, w_qkv, b_qkv, w_proj)
    res = run_bass_kernel_spmd(
        nc,
        in_maps,
        core_ids=list(range(N_CORES)),
        trace=_trace,
        tmpdir=_tmpdir,
    )
    out = np.empty((B, T, C), dtype=np.float32)
    for b in range(B):
        out[b] = (
            res.results[2 * b]["out"].astype(np.float32)
            + res.results[2 * b + 1]["out"].astype(np.float32)
            + b_proj
        )
    kernel._last_result = res
    return out


# revision 31
# speedup vs baseline: 1.0252x; 1.0024x over previous
"""Causal self-attention on 8 Trainium2 NeuronCores.

Problem (hardcoded): B=4, T=2048, C=1024, H=16, D=64.
  qkv = x @ w_qkv + b_qkv ; per-head causal softmax attention ; out = attn @ w_proj + b_proj

Sharding (per hint): tensor-parallel over heads x data-parallel over batch.
  core c -> batch b = c // 2, head group g = c % 2 (heads g*8 .. g*8+7).
Each core computes QKV for its 8 heads, causal attention, and a partial
projection (its 512 input channels of w_proj). Host sums the two partials per
batch and adds b_proj.

v2: virtual-clock emitter. Attention (S -> exp -> PV) is paced by the Scalar
engine's exp; all other PE work (QKV projections, output projection) is
emitted matmul-granular into the predicted exp-stall windows from a dedicated
2-bank PSUM fill pool, so fills never wait on the exp-paced S ring (which
shrinks to 2 slots). Vector load is cut by merging the softmax-denominator
row into the O-tile drain (one copy instead of two), reciprocal in place, and
moving the normalize broadcast+multiply to the (otherwise idle) GpSimd/Pool
engine. PE warmup matmuls run at t=0 against a memset dummy tile so the
p-state ramp happens before real work; first DMA parcels are split so the
initial transfers fan across multiple HW queues.
"""

import numpy as np
import ml_dtypes

B, T, C, H, D = 4, 2048, 1024, 16, 64
HL = H // 2          # heads per core
CL = HL * D          # local channels (512)
NPAIR = HL // 2      # head pairs per core (4)
CCH = C // 128       # contraction chunks for qkv (8)
PCH = CL // 128      # contraction chunks for proj (4)
TT = T // 128        # t tiles (16)
NI = T // 512        # i chunks (4)
N_CORES = 8
BF16 = ml_dtypes.bfloat16

# ---- virtual-clock cost constants (ns) ----
MM = 217.0        # N=512 matmul issue slot
SEM = 150.0       # cross-engine semaphore hop
EXP_FIX = 190.0   # activation fixed overhead
EXP_EL = 1.0 / 1.2
MASK_C = 330.0
OC_C = 700.0
RECIP_C = 560.0
DRAIN_C = 800.0   # [128,512] psum->sbuf drain on DVE
BCAST_C = 650.0
PMUL_C = 1150.0
DMA_BW = 0.0029   # ns per byte (~345 GB/s aggregate)

N_WARMUP = 24     # dummy PE warmup matmuls

_compiled = {}


def _build(nc, bias_zero=False):
    import concourse.tile as tile
    from concourse import mybir

    bf = mybir.dt.bfloat16
    f32 = mybir.dt.float32
    Exp = mybir.ActivationFunctionType.Exp
    Ident = mybir.ActivationFunctionType.Identity

    xT = nc.dram_tensor("xT", [128, 4 * CCH * 512], bf, kind="ExternalInput").ap()
    wq = nc.dram_tensor("wq", [NPAIR, 128, CCH * 128], bf, kind="ExternalInput").ap()
    wk = nc.dram_tensor("wk", [NPAIR, 128, CCH * 128], bf, kind="ExternalInput").ap()
    wv = nc.dram_tensor("wv", [128, CCH * CL], bf, kind="ExternalInput").ap()
    bq = nc.dram_tensor("bq", [128, NPAIR], f32, kind="ExternalInput").ap()
    bk = nc.dram_tensor("bk", [128, NPAIR], f32, kind="ExternalInput").ap()
    bv = nc.dram_tensor("bv", [1, CL], f32, kind="ExternalInput").ap()
    wp = nc.dram_tensor("wp", [CL, C], bf, kind="ExternalInput").ap()
    out = nc.dram_tensor("out", [T, C], bf, kind="ExternalOutput").ap()

    xT_r = xT.rearrange("p (q cc t) -> p q cc t", q=4, cc=CCH)
    wv_r = wv.rearrange("p (cc m) -> p cc m", cc=CCH)
    wp_r = wp.rearrange("(cc p) n -> p cc n", p=128)
    wq_r = wq.rearrange("a p (cc m) -> a p cc m", m=128)
    wk_r = wk.rearrange("a p (cc m) -> a p cc m", m=128)

    with tile.TileContext(nc) as tc:
        import contextlib

        with contextlib.ExitStack() as ctx:
            persist = ctx.enter_context(tc.tile_pool(name="persist", bufs=1))
            # PSUM: s-ring 2x[128,2,512] (4 banks) + o0/o1 (2 banks) +
            # fill pool 2x[128,512] (2 banks) = 8 banks.
            ps_pool = ctx.enter_context(tc.tile_pool(name="ps_pool", bufs=2, space="PSUM"))
            o_ps = ctx.enter_context(tc.tile_pool(name="o_ps", bufs=1, space="PSUM"))
            f_ps = ctx.enter_context(tc.tile_pool(name="f_ps", bufs=2, space="PSUM"))
            p_pool = ctx.enter_context(tc.tile_pool(name="p_pool", bufs=8))
            oc_pool = ctx.enter_context(tc.tile_pool(name="oc_pool", bufs=2))
            rb_pool = ctx.enter_context(tc.tile_pool(name="rb_pool", bufs=2))
            st_pool = ctx.enter_context(tc.tile_pool(name="st_pool", bufs=4))

            # ---- persistent SBUF tensors ----
            xT_sb = persist.tile([128, 4, CCH, 512], bf)
            wq_sb = persist.tile([128, NPAIR, CCH, 128], bf)
            wk_sb = persist.tile([128, NPAIR, CCH, 128], bf)
            wv_sb = persist.tile([128, CCH, CL], bf)
            wp_sb = persist.tile([128, PCH, C], bf)
            bq_sb = persist.tile([128, NPAIR], f32)
            bk_sb = persist.tile([128, NPAIR], f32)
            bv1_sb = persist.tile([1, CL], f32)
            bv_sb = persist.tile([128, CL], f32)
            QT_sb = persist.tile([128, NPAIR, T], bf)
            KT_sb = persist.tile([128, NPAIR, T], bf)
            V_sb = persist.tile([128, TT, HL, 2 * D], bf)
            AT_sb = persist.tile([128, PCH, T], bf)
            tri_sb = persist.tile([128, 2, 128], bf)
            dum_sb = persist.tile([128, 512], bf)

            # ---- DMA emission (sync queue), first parcels split fine ----
            dma_t = [3500.0]
            arrive = {}

            def dstart(name, out_ap, in_ap, nbytes):
                nc.sync.dma_start(out=out_ap, in_=in_ap)
                dma_t[0] = dma_t[0] + 565.0
                t = max(dma_t[0] + 1500.0, arrive.get("_last", 0.0)) + nbytes * DMA_BW
                arrive[name] = t + 1200.0
                arrive["_last"] = t

            dstart("bv", bv1_sb[:], bv[:], 1024)
            # first wave: fan the first four big parcels across four idle
            # engine queues so their HWDGE issues run concurrently
            nc.scalar.dma_start(out=wv_sb[:, 0:2], in_=wv_r[:, 0:2])
            nc.gpsimd.dma_start(out=xT_sb[:, 0, 0:2], in_=xT_r[:, 0, 0:2])
            nc.gpsimd.dma_start(out=wv_sb[:, 2:4], in_=wv_r[:, 2:4])
            arrive["wv0"] = 5600.0
            arrive["x00"] = 5900.0
            arrive["wv1"] = 6400.0
            for h in range(1, 4):
                if h > 1:
                    dstart(f"wv{h}", wv_sb[:, 2 * h : 2 * h + 2], wv_r[:, 2 * h : 2 * h + 2], 262144)
                dstart(f"x0{h}", xT_sb[:, 0, 2 * h : 2 * h + 2], xT_r[:, 0, 2 * h : 2 * h + 2], 262144)
            for p in range(NPAIR):
                dstart(f"wk{p}", wk_sb[:, p], wk_r[p], 262144)
                dstart(f"wq{p}", wq_sb[:, p], wq_r[p], 262144)
            dstart("bq", bq_sb[:], bq[:], 2048)
            dstart("bk", bk_sb[:], bk[:], 2048)
            for q in (1, 2, 3):
                for h in range(2):
                    dstart(f"x{q}{h}", xT_sb[:, q, 4 * h : 4 * h + 4], xT_r[:, q, 4 * h : 4 * h + 4], 524288)
            for cc in range(PCH):
                dstart(f"wp{cc}", wp_sb[:, cc, :], wp_r[:, cc, :], 262144)
            for q in (1, 2, 3):
                arrive[f"x{q}"] = max(arrive[f"x{q}0"], arrive[f"x{q}1"])
            arrive["x0"] = arrive["x03"]

            # setup ops on idle engines
            nc.vector.memset(tri_sb[:], 1.0)
            nc.gpsimd.affine_select(
                out=tri_sb[:],
                in_=tri_sb[:],
                compare_op=mybir.AluOpType.is_ge,
                fill=0.0,
                base=0,
                pattern=[[0, 2], [1, 128]],
                channel_multiplier=-1,
            )
            nc.vector.memset(V_sb[:, :, :, D], 1.0)
            nc.vector.memset(V_sb[:, :, :, D + 1 : 2 * D], 0.0)
            nc.gpsimd.memset(dum_sb[:], 0.5)
            nc.gpsimd.partition_broadcast(bv_sb[:], bv1_sb[:])

            # ---- virtual clocks / completion records ----
            clk = {"pe": 0.0, "sc": 0.0, "dve": 0.0, "pool": 0.0}
            qt_done = {}
            kt_done = {}
            v_done = {}
            at_done = {}

            # ---- PE warmup (dummy matmuls into the o banks) ----
            o0 = o_ps.tile([128, 512], f32, tag="o0")
            o1 = o_ps.tile([128, 512], f32, tag="o1")
            warm_cnt = [0]
            warm_ok = [True]

            def emit_warm():
                nc.tensor.matmul(
                    (o0 if warm_cnt[0] % 2 else o1)[:],
                    lhsT=dum_sb[:, 0:128],
                    rhs=dum_sb[:],
                    start=True,
                    stop=True,
                    skip_group_check=True,
                )
                warm_cnt[0] += 1

            for w in range(N_WARMUP):
                emit_warm()
            clk["pe"] = 600.0 + N_WARMUP * 330.0

            # ---- fill streams ----------------------------------------
            # A stream is a list of steps [ready, kind, emit, cost]; it holds
            # one f_ps slot from its first mm until its dve drain is emitted,
            # so drains are always emitted in-line when popped.

            def v_stream(tt):
                box = {}
                steps = []
                q, off = tt // 4, (tt % 4) * 128

                def mk_mm(cc):
                    def f():
                        if cc == 0:
                            box["ps"] = f_ps.tile([128, 512], f32, tag="f", name="fps")
                        nc.tensor.matmul(
                            box["ps"][:],
                            lhsT=xT_sb[:, q, cc, off : off + 128],
                            rhs=wv_sb[:, cc, :],
                            start=(cc == 0),
                            stop=(cc == CCH - 1),
                            skip_group_check=True,
                        )
                    return f

                for cc in range(CCH):
                    xk = f"x{q}{cc // 2}" if q == 0 else f"x{q}{cc // 4}"
                    rd = max(arrive.get(f"wv{cc // 2}", 0.0), arrive.get(xk, 0.0))
                    steps.append([rd, "mm", mk_mm(cc), MM])

                def drain():
                    if bias_zero and clk["sc"] + 900.0 < clk["pe"] - 700.0:
                        clk["sc"] = max(clk["sc"], clk["pe"]) + 900.0
                        nc.scalar.copy(
                            V_sb[:, tt, :, 0:D],
                            box["ps"][:].rearrange("p (h d) -> p h d", h=HL),
                        )
                        v_done[tt] = clk["sc"] + SEM
                    elif bias_zero:
                        clk["dve"] = max(clk["dve"], clk["pe"]) + DRAIN_C
                        nc.vector.tensor_copy(
                            V_sb[:, tt, :, 0:D],
                            box["ps"][:].rearrange("p (h d) -> p h d", h=HL),
                        )
                        v_done[tt] = clk["dve"] + SEM
                    else:
                        clk["dve"] = max(clk["dve"], clk["pe"]) + DRAIN_C
                        nc.vector.tensor_add(
                            V_sb[:, tt, :, 0:D],
                            box["ps"][:].rearrange("p (h d) -> p h d", h=HL),
                            bv_sb[:].rearrange("p (h d) -> p h d", h=HL),
                        )
                        v_done[tt] = clk["dve"] + SEM
                steps.append([0.0, "none", drain, 0.0])
                return steps

            def qk_stream(pair, which, q):
                box = {}
                steps = []
                w_sb, dst, b_sb, done = (
                    (wq_sb, QT_sb, bq_sb, qt_done),
                    (wk_sb, KT_sb, bk_sb, kt_done),
                )[which]

                def mk_mm(cc):
                    def f():
                        if cc == 0:
                            box["ps"] = f_ps.tile([128, 512], f32, tag="f", name="fps")
                        nc.tensor.matmul(
                            box["ps"][:],
                            lhsT=w_sb[:, pair, cc, :],
                            rhs=xT_sb[:, q, cc, :],
                            start=(cc == 0),
                            stop=(cc == CCH - 1),
                            skip_group_check=True,
                        )
                    return f

                wname = f"w{'qk'[which]}{pair}"
                for cc in range(CCH):
                    rd = max(arrive.get(wname, 0.0), arrive.get(f"x{q}", 0.0))
                    steps.append([rd, "mm", mk_mm(cc), MM])

                def drain():
                    if bias_zero and clk["sc"] + 800.0 < clk["pe"] - 700.0:
                        clk["sc"] = max(clk["sc"], clk["pe"]) + 800.0
                        nc.scalar.copy(
                            dst[:, pair, q * 512 : (q + 1) * 512], box["ps"][:]
                        )
                        done[(pair, q)] = clk["sc"] + SEM
                    elif bias_zero:
                        clk["dve"] = max(clk["dve"], clk["pe"]) + DRAIN_C
                        nc.vector.tensor_copy(
                            dst[:, pair, q * 512 : (q + 1) * 512], box["ps"][:]
                        )
                        done[(pair, q)] = clk["dve"] + SEM
                    else:
                        clk["dve"] = max(clk["dve"], clk["pe"]) + DRAIN_C
                        nc.vector.tensor_scalar_add(
                            dst[:, pair, q * 512 : (q + 1) * 512],
                            box["ps"][:],
                            b_sb[:, pair : pair + 1],
                        )
                        done[(pair, q)] = clk["dve"] + SEM
                steps.append([0.0, "none", drain, 0.0])
                return steps

            def proj_stream(tt, nh, so_box):
                ci = tt // 4
                box = {}
                steps = []

                def mk_mm(cc):
                    def f():
                        if cc == 0:
                            if nh == 0:
                                so_box["t"] = st_pool.tile(
                                    [128, 1024], bf, tag="so", name="so"
                                )
                            if attn_done[0]:
                                pool_, tag_ = tail_slots[tail_cnt[0] % len(tail_slots)]
                                tail_cnt[0] += 1
                            else:
                                pool_, tag_ = f_ps, "f"
                            box["ps"] = pool_.tile([128, 512], f32, tag=tag_, name="fps")
                        nc.tensor.matmul(
                            box["ps"][:],
                            lhsT=AT_sb[:, cc, tt * 128 : (tt + 1) * 128],
                            rhs=wp_sb[:, cc, nh * 512 : (nh + 1) * 512],
                            start=(cc == 0),
                            stop=(cc == PCH - 1),
                            skip_group_check=True,
                        )
                    return f

                for cc in range(PCH):
                    steps.append([("at", cc, ci), "mm", mk_mm(cc), MM])

                def drain_v():
                    so = so_box["t"]
                    nc.vector.tensor_copy(so[:, nh * 512 : (nh + 1) * 512], box["ps"][:])
                    if ci == NI - 1:
                        nc.sync.dma_start(
                            out=out[tt * 128 : (tt + 1) * 128, nh * 512 : (nh + 1) * 512],
                            in_=so[:, nh * 512 : (nh + 1) * 512],
                        )
                    elif nh == 1:
                        nc.sync.dma_start(out=out[tt * 128 : (tt + 1) * 128, :], in_=so[:])

                def drain_s():
                    so = so_box["t"]
                    nc.scalar.copy(so[:, nh * 512 : (nh + 1) * 512], box["ps"][:])
                    if ci == NI - 1:
                        nc.sync.dma_start(
                            out=out[tt * 128 : (tt + 1) * 128, nh * 512 : (nh + 1) * 512],
                            in_=so[:, nh * 512 : (nh + 1) * 512],
                        )
                    elif nh == 1:
                        nc.sync.dma_start(out=out[tt * 128 : (tt + 1) * 128, :], in_=so[:])

                def drain():
                    if attn_done[0] and tail_cnt[0] % 2:
                        clk["sc"] = max(clk["sc"], clk["pe"]) + 650.0
                        drain_s()
                    else:
                        clk["dve"] = max(clk["dve"], clk["pe"]) + DRAIN_C
                        drain_v()
                steps.append([0.0, "none", drain, 0.0])
                return steps

            streams = []
            key_of = {}

            def add(key, st):
                key_of[id(st)] = key
                streams.append(st)

            for tt in range(4):
                add(("v", tt), v_stream(tt))
            for p in range(NPAIR):
                add(("k", p, 0), qk_stream(p, 1, 0))
                add(("q", p, 0), qk_stream(p, 0, 0))
            for tt in range(4, 8):
                add(("v", tt), v_stream(tt))
            for p in range(NPAIR):
                add(("k", p, 1), qk_stream(p, 1, 1))
                add(("q", p, 1), qk_stream(p, 0, 1))
            for tt in range(8, 12):
                add(("v", tt), v_stream(tt))
            for p in range(NPAIR):
                add(("k", p, 2), qk_stream(p, 1, 2))
                add(("q", p, 2), qk_stream(p, 0, 2))
            for tt in range(12, 16):
                add(("v", tt), v_stream(tt))
            for p in range(NPAIR):
                add(("k", p, 3), qk_stream(p, 1, 3))
                add(("q", p, 3), qk_stream(p, 0, 3))
            for tt in range(TT):
                sb = {}
                add(("pj", tt, 0), proj_stream(tt, 0, sb))
                add(("pj", tt, 1), proj_stream(tt, 1, sb))

            active = []
            max_active = [2]
            attn_done = [False]
            tail_slots = [(f_ps, "f"), (ps_pool, "s"), (o_ps, "o0"), (f_ps, "f"), (ps_pool, "s"), (o_ps, "o1")]
            tail_cnt = [0]

            def refill_active():
                while len(active) < max_active[0] and streams:
                    active.append(streams.pop(0))

            def step_ready(st):
                r = st[0][0]
                if isinstance(r, tuple):
                    return at_done.get((r[1], r[2]), None)
                return r

            def exec_step(st):
                r, kind, emit, cost = st.pop(0)
                if isinstance(r, tuple):
                    r = at_done.get((r[1], r[2]), 0.0)
                if kind == "mm":
                    clk["pe"] = max(clk["pe"], r) + cost
                    emit()
                elif kind == "sc":
                    clk["sc"] = max(clk["sc"], clk["pe"]) + cost
                    emit()
                elif kind == "none":
                    emit()
                else:
                    clk["dve"] = max(clk["dve"], clk["pe"]) + cost
                    emit()
                if not st:
                    active.remove(st)
                    refill_active()

            def pump_fills(target, allow_drain=True):
                refill_active()
                while clk["pe"] + MM <= target:
                    pick = None
                    for st in active:
                        rd = step_ready(st)
                        if rd is None:
                            continue
                        if st[0][1] in ("dve", "sc", "none"):
                            if allow_drain:
                                pick = st
                                break
                            continue
                        if rd <= max(clk["pe"] + 120.0, target - MM):
                            pick = st
                            break
                    if pick is None:
                        if warm_ok[0] and warm_cnt[0] < 90 and active:
                            nxt = min(
                                (step_ready(st) for st in active if step_ready(st) is not None),
                                default=None,
                            )
                            if nxt is None:
                                return
                            while clk["pe"] + MM < min(nxt, target) and warm_cnt[0] < 90:
                                emit_warm()
                                clk["pe"] = clk["pe"] + MM
                            if clk["pe"] + MM > target:
                                return
                            continue
                        return
                    exec_step(pick)

            def ensure(pred):
                # force-run streams in order until pred() holds
                guard = 0
                while not pred() and guard < 10000:
                    guard += 1
                    refill_active()
                    picked = False
                    for st in active:
                        rd = step_ready(st)
                        if rd is not None:
                            exec_step(st)
                            picked = True
                            break
                    if not picked:
                        if active:
                            exec_step(active[0])
                        else:
                            break

            # ---- attention ----------------------------------------------
            slot_ring = []     # global exp-end ring (s pool bufs=2)
            o_free = [clk["pe"]]
            deferred = []      # deferred recip ops (DVE)

            def emit_chunk(pair, ci):
                njt = 4 * (ci + 1)
                ensure(lambda: (pair, ci) in qt_done)
                for q in range(ci + 1):
                    ensure(lambda q=q: (pair, q) in kt_done)
                ensure(lambda: njt - 1 in v_done)
                pt_tiles = {}

                def emit_S(jt):
                    rd = max(
                        qt_done.get((pair, ci), 0.0),
                        kt_done.get((pair, jt // 4), 0.0),
                        v_done.get(jt, 0.0),
                    )
                    if len(slot_ring) >= 2:
                        rd = max(rd, slot_ring[-2])
                    r = jt - 4 * ci
                    pump_fills(rd, allow_drain=(r < -1 or jt < 2))
                    st = ps_pool.tile([128, 2, 512], f32, tag="s")
                    i0 = max(0, 128 * r)
                    for s in range(2):
                        nc.tensor.matmul(
                            st[:, s, i0:512],
                            lhsT=KT_sb[64 * s : 64 * (s + 1), pair, jt * 128 : (jt + 1) * 128],
                            rhs=QT_sb[64 * s : 64 * (s + 1), pair, ci * 512 + i0 : (ci + 1) * 512],
                            start=True,
                            stop=True,
                            skip_group_check=True,
                        )
                    clk["pe"] = max(clk["pe"], rd) + 320.0
                    pt = p_pool.tile([128, 2, 512], bf, tag="p")
                    nc.scalar.activation(pt[:, :, i0:512], st[:, :, i0:512], Exp, scale=0.125)
                    ap = 2 * (512 - i0)
                    clk["sc"] = max(clk["sc"], clk["pe"] + SEM) + EXP_FIX + ap * EXP_EL
                    slot_ring.append(clk["sc"])
                    if r >= 0:
                        nc.vector.tensor_mul(
                            pt[:, :, i0 : i0 + 128], pt[:, :, i0 : i0 + 128], tri_sb[:]
                        )
                        clk["dve"] = max(clk["dve"], clk["sc"] + SEM) + MASK_C
                        pt_ready = clk["dve"] + SEM
                    else:
                        pt_ready = clk["sc"] + SEM
                    pt_tiles[jt] = (pt, i0, pt_ready)
                    if jt == 1 and deferred:
                        for f in deferred:
                            f()
                        deferred.clear()

                def emit_PV(jt):
                    warm_ok[0] = False
                    pt, i0, rdy = pt_tiles.pop(jt)
                    if jt == 0:
                        rdy = max(rdy, o_free[0])
                    pump_fills(rdy, allow_drain=(jt < 4 * ci - 1))
                    for s, ot in enumerate((o0, o1)):
                        nc.tensor.matmul(
                            ot[:, i0:512],
                            lhsT=V_sb[:, jt, 2 * pair + s, :],
                            rhs=pt[:, s, i0:512],
                            start=(jt == 0),
                            stop=(jt == njt - 1),
                            skip_group_check=True,
                        )
                    clk["pe"] = max(clk["pe"], rdy) + 2 * 240.0

                emit_S(0)
                for jt in range(1, njt):
                    emit_S(jt)
                    emit_PV(jt - 1)
                emit_PV(njt - 1)

                # ---- normalize ----
                oc0 = oc_pool.tile([128, 512], f32, tag="oc0")
                oc1 = oc_pool.tile([128, 512], f32, tag="oc1")
                dn0 = rb_pool.tile([1, 512], f32, tag="dn0")
                dn1 = rb_pool.tile([1, 512], f32, tag="dn1")
                nc.vector.tensor_copy(oc0[0:D, :], o0[0:D, :])
                nc.vector.tensor_copy(dn0[:], o0[D : D + 1, :])
                if (pair == NPAIR - 1 and ci == NI - 1) or clk["sc"] + 1400.0 < clk["pe"] - 900.0:
                    nc.scalar.copy(oc1[D : 2 * D, :], o1[0:D, :])
                    nc.scalar.copy(dn1[:], o1[D : D + 1, :])
                    clk["sc"] = max(clk["sc"], clk["pe"]) + 1300.0
                    clk["dve"] = max(clk["dve"], clk["pe"] + SEM) + OC_C + RECIP_C
                    o_free[0] = max(clk["dve"], clk["sc"])
                else:
                    nc.vector.tensor_copy(oc1[D : 2 * D, :], o1[0:D, :])
                    nc.vector.tensor_copy(dn1[:], o1[D : D + 1, :])
                    clk["dve"] = max(clk["dve"], clk["pe"] + SEM) + 2 * OC_C + 2 * RECIP_C
                    o_free[0] = clk["dve"]

                last = pair == NPAIR - 1 and ci == NI - 1

                def norm_tail(pair=pair, ci=ci, oc0=oc0, oc1=oc1, dn0=dn0, dn1=dn1, last=last):
                    rc = rb_pool.tile([1, 1024], f32, tag="rc")
                    rb = rb_pool.tile([128, 1024], f32, tag="rb")
                    if last:
                        # per-head pipeline: head0 mul overlaps head1 chain
                        nc.vector.reciprocal_approx_fast(rc[0:1, 0:512], dn0[:])
                        nc.gpsimd.partition_broadcast(rb[:, 0:512], rc[0:1, 0:512])
                        nc.vector.reciprocal_approx_fast(rc[0:1, 512:1024], dn1[:])
                        nc.gpsimd.partition_broadcast(rb[:, 512:1024], rc[0:1, 512:1024])
                        nc.vector.tensor_mul(
                            AT_sb[0:D, pair, ci * 512 : (ci + 1) * 512],
                            oc0[0:D, :],
                            rb[0:D, 0:512],
                        )
                        nc.vector.tensor_mul(
                            AT_sb[D : 2 * D, pair, ci * 512 : (ci + 1) * 512],
                            oc1[D : 2 * D, :],
                            rb[D : 2 * D, 512:1024],
                        )
                        clk["dve"] = max(clk["dve"], clk["pool"]) + 2 * RECIP_C + 2 * 900.0
                        at_done[(pair, ci)] = clk["dve"] + SEM
                        return
                    nc.vector.reciprocal_approx_fast(rc[0:1, 0:512], dn0[:])
                    nc.vector.reciprocal_approx_fast(rc[0:1, 512:1024], dn1[:])
                    clk["dve"] = clk["dve"] + 2 * RECIP_C
                    nc.gpsimd.partition_broadcast(rb[:], rc[:])
                    clk["pool"] = max(clk["pool"], clk["dve"] + SEM) + 2 * BCAST_C
                    nc.vector.tensor_mul(
                        AT_sb[0:D, pair, ci * 512 : (ci + 1) * 512],
                        oc0[0:D, :],
                        rb[0:D, 0:512],
                    )
                    nc.vector.tensor_mul(
                        AT_sb[D : 2 * D, pair, ci * 512 : (ci + 1) * 512],
                        oc1[D : 2 * D, :],
                        rb[D : 2 * D, 512:1024],
                    )
                    clk["dve"] = max(clk["dve"], clk["pool"] + SEM) + 2 * 900.0
                    at_done[(pair, ci)] = clk["dve"] + SEM

                deferred.append(norm_tail)

            for ci in range(NI):
                for pair in range(NPAIR):
                    emit_chunk(pair, ci)
            for f in deferred:
                f()
            deferred.clear()

            # ---- tail: drain all remaining fills ----
            attn_done[0] = True
            max_active[0] = 6
            refill_active()
            guard = 0
            while (active or streams) and guard < 20000:
                guard += 1
                refill_active()
                picked = False
                for st in active:
                    if step_ready(st) is not None:
                        exec_step(st)
                        picked = True
                        break
                if not picked and active:
                    exec_step(active[0])
    return nc


def _get_compiled(bias_zero):
    if bias_zero not in _compiled:
        from concourse import bacc

        nc = bacc.Bacc(
            "TRN2", target_bir_lowering=False, debug=False, num_devices=N_CORES
        )
        _build(nc, bias_zero)
        nc.compile()
        _compiled[bias_zero] = nc
    return _compiled[bias_zero]


def _shard_inputs(x, w_qkv, b_qkv, w_proj):
    """Build the 8 per-core input dicts (host-side transpose/slice/cast)."""
    in_maps = []
    wq_f, wk_f, wv_f = w_qkv[:, :C], w_qkv[:, C : 2 * C], w_qkv[:, 2 * C :]
    for c in range(N_CORES):
        b, g = c // 2, c % 2
        sl = slice(g * CL, (g + 1) * CL)
        bqs = np.ascontiguousarray(b_qkv[0 * C :][sl].reshape(NPAIR, 128).T)
        bks = np.ascontiguousarray(b_qkv[1 * C :][sl].reshape(NPAIR, 128).T)
        bvs = np.ascontiguousarray(b_qkv[2 * C :][sl][None, :])
        wq_p = np.ascontiguousarray(
            wq_f[:, sl].reshape(CCH, 128, NPAIR, 128).transpose(2, 1, 0, 3)
            .reshape(NPAIR, 128, CCH * 128)
        )
        wk_p = np.ascontiguousarray(
            wk_f[:, sl].reshape(CCH, 128, NPAIR, 128).transpose(2, 1, 0, 3)
            .reshape(NPAIR, 128, CCH * 128)
        )
        xT_p = np.ascontiguousarray(
            x[b].T.reshape(CCH, 128, 4, 512).transpose(1, 2, 0, 3)
            .reshape(128, 4 * CCH * 512)
        )
        wv_p = np.ascontiguousarray(
            wv_f[:, sl].reshape(CCH, 128, CL).transpose(1, 0, 2)
            .reshape(128, CCH * CL)
        )
        in_maps.append(
            {
                "xT": xT_p.astype(BF16),
                "wq": wq_p.astype(BF16),
                "wk": wk_p.astype(BF16),
                "wv": wv_p.astype(BF16),
                "bq": bqs.astype(np.float32),
                "bk": bks.astype(np.float32),
                "bv": bvs.astype(np.float32),
                "wp": np.ascontiguousarray(w_proj[sl, :]).astype(BF16),
            }
        )
    return in_maps


def kernel(x, w_qkv, b_qkv, w_proj, b_proj, _trace=False, _tmpdir=None):
    from concourse.bass_utils import run_bass_kernel_spmd

    x = np.asarray(x, dtype=np.float32)
    w_qkv = np.asarray(w_qkv, dtype=np.float32)
    b_qkv = np.asarray(b_qkv, dtype=np.float32)
    w_proj = np.asarray(w_proj, dtype=np.float32)
    b_proj = np.asarray(b_proj, dtype=np.float32)

    nc = _get_compiled(not b_qkv.any())
    in_maps = _shard_inputs(x, w_qkv, b_qkv, w_proj)
    res = run_bass_kernel_spmd(
        nc,
        in_maps,
        core_ids=list(range(N_CORES)),
        trace=_trace,
        tmpdir=_tmpdir,
    )
    out = np.empty((B, T, C), dtype=np.float32)
    for b in range(B):
        out[b] = (
            res.results[2 * b]["out"].astype(np.float32)
            + res.results[2 * b + 1]["out"].astype(np.float32)
            + b_proj
        )
    kernel._last_result = res
    return out


# revision 32
# speedup vs baseline: 1.0318x; 1.0065x over previous
"""Causal self-attention on 8 Trainium2 NeuronCores.

Problem (hardcoded): B=4, T=2048, C=1024, H=16, D=64.
  qkv = x @ w_qkv + b_qkv ; per-head causal softmax attention ; out = attn @ w_proj + b_proj

Sharding (per hint): tensor-parallel over heads x data-parallel over batch.
  core c -> batch b = c // 2, head group g = c % 2 (heads g*8 .. g*8+7).
Each core computes QKV for its 8 heads, causal attention, and a partial
projection (its 512 input channels of w_proj). Host sums the two partials per
batch and adds b_proj.

v2: virtual-clock emitter. Attention (S -> exp -> PV) is paced by the Scalar
engine's exp; all other PE work (QKV projections, output projection) is
emitted matmul-granular into the predicted exp-stall windows from a dedicated
2-bank PSUM fill pool, so fills never wait on the exp-paced S ring (which
shrinks to 2 slots). Vector load is cut by merging the softmax-denominator
row into the O-tile drain (one copy instead of two), reciprocal in place, and
moving the normalize broadcast+multiply to the (otherwise idle) GpSimd/Pool
engine. PE warmup matmuls run at t=0 against a memset dummy tile so the
p-state ramp happens before real work; first DMA parcels are split so the
initial transfers fan across multiple HW queues.
"""

import numpy as np
import ml_dtypes

B, T, C, H, D = 4, 2048, 1024, 16, 64
HL = H // 2          # heads per core
CL = HL * D          # local channels (512)
NPAIR = HL // 2      # head pairs per core (4)
CCH = C // 128       # contraction chunks for qkv (8)
PCH = CL // 128      # contraction chunks for proj (4)
TT = T // 128        # t tiles (16)
NI = T // 512        # i chunks (4)
N_CORES = 8
BF16 = ml_dtypes.bfloat16

# ---- virtual-clock cost constants (ns) ----
MM = 217.0        # N=512 matmul issue slot
SEM = 150.0       # cross-engine semaphore hop
EXP_FIX = 190.0   # activation fixed overhead
EXP_EL = 1.0 / 1.2
MASK_C = 330.0
OC_C = 700.0
RECIP_C = 560.0
DRAIN_C = 800.0   # [128,512] psum->sbuf drain on DVE
BCAST_C = 650.0
PMUL_C = 1150.0
DMA_BW = 0.0029   # ns per byte (~345 GB/s aggregate)

N_WARMUP = 24     # dummy PE warmup matmuls

_compiled = {}


def _build(nc, bias_zero=False):
    import concourse.tile as tile
    from concourse import mybir

    bf = mybir.dt.bfloat16
    f32 = mybir.dt.float32
    Exp = mybir.ActivationFunctionType.Exp
    Ident = mybir.ActivationFunctionType.Identity

    xT = nc.dram_tensor("xT", [128, 4 * CCH * 512], bf, kind="ExternalInput").ap()
    wq = nc.dram_tensor("wq", [NPAIR, 128, CCH * 128], bf, kind="ExternalInput").ap()
    wk = nc.dram_tensor("wk", [NPAIR, 128, CCH * 128], bf, kind="ExternalInput").ap()
    wv = nc.dram_tensor("wv", [128, CCH * CL], bf, kind="ExternalInput").ap()
    bq = nc.dram_tensor("bq", [128, NPAIR], f32, kind="ExternalInput").ap()
    bk = nc.dram_tensor("bk", [128, NPAIR], f32, kind="ExternalInput").ap()
    bv = nc.dram_tensor("bv", [1, CL], f32, kind="ExternalInput").ap()
    wp = nc.dram_tensor("wp", [CL, C], bf, kind="ExternalInput").ap()
    out = nc.dram_tensor("out", [T, C], bf, kind="ExternalOutput").ap()

    xT_r = xT.rearrange("p (q cc t) -> p q cc t", q=4, cc=CCH)
    wv_r = wv.rearrange("p (cc m) -> p cc m", cc=CCH)
    wp_r = wp.rearrange("(cc p) n -> p cc n", p=128)
    wq_r = wq.rearrange("a p (cc m) -> a p cc m", m=128)
    wk_r = wk.rearrange("a p (cc m) -> a p cc m", m=128)

    with tile.TileContext(nc) as tc:
        import contextlib

        with contextlib.ExitStack() as ctx:
            persist = ctx.enter_context(tc.tile_pool(name="persist", bufs=1))
            # PSUM: s-ring 2x[128,2,512] (4 banks) + o0/o1 (2 banks) +
            # fill pool 2x[128,512] (2 banks) = 8 banks.
            ps_pool = ctx.enter_context(tc.tile_pool(name="ps_pool", bufs=2, space="PSUM"))
            o_ps = ctx.enter_context(tc.tile_pool(name="o_ps", bufs=1, space="PSUM"))
            f_ps = ctx.enter_context(tc.tile_pool(name="f_ps", bufs=2, space="PSUM"))
            p_pool = ctx.enter_context(tc.tile_pool(name="p_pool", bufs=8))
            oc_pool = ctx.enter_context(tc.tile_pool(name="oc_pool", bufs=2))
            rb_pool = ctx.enter_context(tc.tile_pool(name="rb_pool", bufs=2))
            st_pool = ctx.enter_context(tc.tile_pool(name="st_pool", bufs=4))

            # ---- persistent SBUF tensors ----
            xT_sb = persist.tile([128, 4, CCH, 512], bf)
            wq_sb = persist.tile([128, NPAIR, CCH, 128], bf)
            wk_sb = persist.tile([128, NPAIR, CCH, 128], bf)
            wv_sb = persist.tile([128, CCH, CL], bf)
            wp_sb = persist.tile([128, PCH, C], bf)
            bq_sb = persist.tile([128, NPAIR], f32)
            bk_sb = persist.tile([128, NPAIR], f32)
            bv1_sb = persist.tile([1, CL], f32)
            bv_sb = persist.tile([128, CL], f32)
            QT_sb = persist.tile([128, NPAIR, T], bf)
            KT_sb = persist.tile([128, NPAIR, T], bf)
            V_sb = persist.tile([128, TT, HL, 2 * D], bf)
            AT_sb = persist.tile([128, PCH, T], bf)
            tri_sb = persist.tile([128, 2, 128], bf)
            dum_sb = persist.tile([128, 512], bf)

            # ---- DMA emission (sync queue), first parcels split fine ----
            dma_t = [3500.0]
            arrive = {}

            def dstart(name, out_ap, in_ap, nbytes):
                nc.sync.dma_start(out=out_ap, in_=in_ap)
                dma_t[0] = dma_t[0] + 565.0
                t = max(dma_t[0] + 1500.0, arrive.get("_last", 0.0)) + nbytes * DMA_BW
                arrive[name] = t + 1200.0
                arrive["_last"] = t

            dstart("bv", bv1_sb[:], bv[:], 1024)
            # first wave: fan the first four big parcels across four idle
            # engine queues so their HWDGE issues run concurrently
            nc.scalar.dma_start(out=wv_sb[:, 0:2], in_=wv_r[:, 0:2])
            nc.gpsimd.dma_start(out=xT_sb[:, 0, 0:2], in_=xT_r[:, 0, 0:2])
            nc.gpsimd.dma_start(out=wv_sb[:, 2:4], in_=wv_r[:, 2:4])
            arrive["wv0"] = 5600.0
            arrive["x00"] = 5900.0
            arrive["wv1"] = 6400.0
            for h in range(1, 4):
                if h > 1:
                    dstart(f"wv{h}", wv_sb[:, 2 * h : 2 * h + 2], wv_r[:, 2 * h : 2 * h + 2], 262144)
                dstart(f"x0{h}", xT_sb[:, 0, 2 * h : 2 * h + 2], xT_r[:, 0, 2 * h : 2 * h + 2], 262144)
            for p in range(NPAIR):
                dstart(f"wk{p}", wk_sb[:, p], wk_r[p], 262144)
                dstart(f"wq{p}", wq_sb[:, p], wq_r[p], 262144)
            dstart("bq", bq_sb[:], bq[:], 2048)
            dstart("bk", bk_sb[:], bk[:], 2048)
            for q in (1, 2, 3):
                for h in range(2):
                    dstart(f"x{q}{h}", xT_sb[:, q, 4 * h : 4 * h + 4], xT_r[:, q, 4 * h : 4 * h + 4], 524288)
            for cc in range(PCH):
                dstart(f"wp{cc}", wp_sb[:, cc, :], wp_r[:, cc, :], 262144)
            for q in (1, 2, 3):
                arrive[f"x{q}"] = max(arrive[f"x{q}0"], arrive[f"x{q}1"])
            arrive["x0"] = arrive["x03"]

            # setup ops on idle engines
            nc.vector.memset(tri_sb[:], 1.0)
            nc.gpsimd.affine_select(
                out=tri_sb[:],
                in_=tri_sb[:],
                compare_op=mybir.AluOpType.is_ge,
                fill=0.0,
                base=0,
                pattern=[[0, 2], [1, 128]],
                channel_multiplier=-1,
            )
            nc.vector.memset(V_sb[:, :, :, D], 1.0)
            nc.vector.memset(V_sb[:, :, :, D + 1 : 2 * D], 0.0)
            nc.gpsimd.memset(dum_sb[:], 0.5)
            nc.gpsimd.partition_broadcast(bv_sb[:], bv1_sb[:])

            # ---- virtual clocks / completion records ----
            clk = {"pe": 0.0, "sc": 0.0, "dve": 0.0, "pool": 0.0}
            qt_done = {}
            kt_done = {}
            v_done = {}
            at_done = {}

            # ---- PE warmup (dummy matmuls into the o banks) ----
            o0 = o_ps.tile([128, 512], f32, tag="o0")
            o1 = o_ps.tile([128, 512], f32, tag="o1")
            warm_cnt = [0]
            warm_ok = [True]

            def emit_warm():
                nc.tensor.matmul(
                    (o0 if warm_cnt[0] % 2 else o1)[:],
                    lhsT=dum_sb[:, 0:128],
                    rhs=dum_sb[:],
                    start=True,
                    stop=True,
                    skip_group_check=True,
                )
                warm_cnt[0] += 1

            for w in range(N_WARMUP):
                emit_warm()
            clk["pe"] = 600.0 + N_WARMUP * 330.0

            # ---- fill streams ----------------------------------------
            # A stream is a list of steps [ready, kind, emit, cost]; it holds
            # one f_ps slot from its first mm until its dve drain is emitted,
            # so drains are always emitted in-line when popped.

            def v_stream(tt):
                box = {}
                steps = []
                q, off = tt // 4, (tt % 4) * 128

                def mk_mm(cc):
                    def f():
                        if cc == 0:
                            box["ps"] = f_ps.tile([128, 512], f32, tag="f", name="fps")
                        nc.tensor.matmul(
                            box["ps"][:],
                            lhsT=xT_sb[:, q, cc, off : off + 128],
                            rhs=wv_sb[:, cc, :],
                            start=(cc == 0),
                            stop=(cc == CCH - 1),
                            skip_group_check=True,
                        )
                    return f

                for cc in range(CCH):
                    xk = f"x{q}{cc // 2}" if q == 0 else f"x{q}{cc // 4}"
                    rd = max(arrive.get(f"wv{cc // 2}", 0.0), arrive.get(xk, 0.0))
                    steps.append([rd, "mm", mk_mm(cc), MM])

                def drain():
                    if bias_zero and clk["sc"] + 900.0 < clk["pe"] - 700.0:
                        clk["sc"] = max(clk["sc"], clk["pe"]) + 900.0
                        nc.scalar.copy(
                            V_sb[:, tt, :, 0:D],
                            box["ps"][:].rearrange("p (h d) -> p h d", h=HL),
                        )
                        v_done[tt] = clk["sc"] + SEM
                    elif bias_zero:
                        clk["dve"] = max(clk["dve"], clk["pe"]) + DRAIN_C
                        nc.vector.tensor_copy(
                            V_sb[:, tt, :, 0:D],
                            box["ps"][:].rearrange("p (h d) -> p h d", h=HL),
                        )
                        v_done[tt] = clk["dve"] + SEM
                    else:
                        clk["dve"] = max(clk["dve"], clk["pe"]) + DRAIN_C
                        nc.vector.tensor_add(
                            V_sb[:, tt, :, 0:D],
                            box["ps"][:].rearrange("p (h d) -> p h d", h=HL),
                            bv_sb[:].rearrange("p (h d) -> p h d", h=HL),
                        )
                        v_done[tt] = clk["dve"] + SEM
                steps.append([0.0, "none", drain, 0.0])
                return steps

            def qk_stream(pair, which, q):
                box = {}
                steps = []
                w_sb, dst, b_sb, done = (
                    (wq_sb, QT_sb, bq_sb, qt_done),
                    (wk_sb, KT_sb, bk_sb, kt_done),
                )[which]

                def mk_mm(cc):
                    def f():
                        if cc == 0:
                            box["ps"] = f_ps.tile([128, 512], f32, tag="f", name="fps")
                        nc.tensor.matmul(
                            box["ps"][:],
                            lhsT=w_sb[:, pair, cc, :],
                            rhs=xT_sb[:, q, cc, :],
                            start=(cc == 0),
                            stop=(cc == CCH - 1),
                            skip_group_check=True,
                        )
                    return f

                wname = f"w{'qk'[which]}{pair}"
                for cc in range(CCH):
                    rd = max(arrive.get(wname, 0.0), arrive.get(f"x{q}", 0.0))
                    steps.append([rd, "mm", mk_mm(cc), MM])

                def drain():
                    if bias_zero and clk["sc"] + 800.0 < clk["pe"] - 700.0:
                        clk["sc"] = max(clk["sc"], clk["pe"]) + 800.0
                        nc.scalar.copy(
                            dst[:, pair, q * 512 : (q + 1) * 512], box["ps"][:]
                        )
                        done[(pair, q)] = clk["sc"] + SEM
                    elif bias_zero:
                        clk["dve"] = max(clk["dve"], clk["pe"]) + DRAIN_C
                        nc.vector.tensor_copy(
                            dst[:, pair, q * 512 : (q + 1) * 512], box["ps"][:]
                        )
                        done[(pair, q)] = clk["dve"] + SEM
                    else:
                        clk["dve"] = max(clk["dve"], clk["pe"]) + DRAIN_C
                        nc.vector.tensor_scalar_add(
                            dst[:, pair, q * 512 : (q + 1) * 512],
                            box["ps"][:],
                            b_sb[:, pair : pair + 1],
                        )
                        done[(pair, q)] = clk["dve"] + SEM
                steps.append([0.0, "none", drain, 0.0])
                return steps

            def proj_stream(tt, nh, so_box):
                ci = tt // 4
                box = {}
                steps = []

                def mk_mm(cc):
                    def f():
                        if cc == 0:
                            if nh == 0:
                                so_box["t"] = st_pool.tile(
                                    [128, 1024], bf, tag="so", name="so"
                                )
                            if attn_done[0]:
                                pool_, tag_ = tail_slots[tail_cnt[0] % len(tail_slots)]
                                tail_cnt[0] += 1
                            else:
                                pool_, tag_ = f_ps, "f"
                            box["ps"] = pool_.tile([128, 512], f32, tag=tag_, name="fps")
                        nc.tensor.matmul(
                            box["ps"][:],
                            lhsT=AT_sb[:, cc, tt * 128 : (tt + 1) * 128],
                            rhs=wp_sb[:, cc, nh * 512 : (nh + 1) * 512],
                            start=(cc == 0),
                            stop=(cc == PCH - 1),
                            skip_group_check=True,
                        )
                    return f

                for cc in range(PCH):
                    steps.append([("at", cc, ci), "mm", mk_mm(cc), MM])

                def drain_v():
                    so = so_box["t"]
                    nc.vector.tensor_copy(so[:, nh * 512 : (nh + 1) * 512], box["ps"][:])
                    if ci == NI - 1:
                        nc.sync.dma_start(
                            out=out[tt * 128 : (tt + 1) * 128, nh * 512 : (nh + 1) * 512],
                            in_=so[:, nh * 512 : (nh + 1) * 512],
                        )
                    elif nh == 1:
                        nc.sync.dma_start(out=out[tt * 128 : (tt + 1) * 128, :], in_=so[:])

                def drain_s():
                    so = so_box["t"]
                    nc.scalar.copy(so[:, nh * 512 : (nh + 1) * 512], box["ps"][:])
                    if ci == NI - 1:
                        nc.sync.dma_start(
                            out=out[tt * 128 : (tt + 1) * 128, nh * 512 : (nh + 1) * 512],
                            in_=so[:, nh * 512 : (nh + 1) * 512],
                        )
                    elif nh == 1:
                        nc.sync.dma_start(out=out[tt * 128 : (tt + 1) * 128, :], in_=so[:])

                def drain():
                    if attn_done[0] and tail_cnt[0] % 2:
                        clk["sc"] = max(clk["sc"], clk["pe"]) + 650.0
                        drain_s()
                    else:
                        clk["dve"] = max(clk["dve"], clk["pe"]) + DRAIN_C
                        drain_v()
                steps.append([0.0, "none", drain, 0.0])
                return steps

            streams = []
            key_of = {}

            def add(key, st):
                key_of[id(st)] = key
                streams.append(st)

            for tt in range(4):
                add(("v", tt), v_stream(tt))
            for p in range(NPAIR):
                add(("k", p, 0), qk_stream(p, 1, 0))
                add(("q", p, 0), qk_stream(p, 0, 0))
            for tt in range(4, 8):
                add(("v", tt), v_stream(tt))
            for p in range(NPAIR):
                add(("k", p, 1), qk_stream(p, 1, 1))
                add(("q", p, 1), qk_stream(p, 0, 1))
            for tt in range(8, 12):
                add(("v", tt), v_stream(tt))
            for p in range(NPAIR):
                add(("k", p, 2), qk_stream(p, 1, 2))
                add(("q", p, 2), qk_stream(p, 0, 2))
            for tt in range(12, 16):
                add(("v", tt), v_stream(tt))
            for p in range(NPAIR):
                add(("k", p, 3), qk_stream(p, 1, 3))
                add(("q", p, 3), qk_stream(p, 0, 3))
            for tt in range(TT):
                sb = {}
                add(("pj", tt, 0), proj_stream(tt, 0, sb))
                add(("pj", tt, 1), proj_stream(tt, 1, sb))

            active = []
            max_active = [2]
            attn_done = [False]
            tail_slots = [(f_ps, "f"), (ps_pool, "s"), (o_ps, "o0"), (f_ps, "f"), (ps_pool, "s"), (o_ps, "o1")]
            tail_cnt = [0]

            def refill_active():
                while len(active) < max_active[0] and streams:
                    active.append(streams.pop(0))

            def step_ready(st):
                r = st[0][0]
                if isinstance(r, tuple):
                    return at_done.get((r[1], r[2]), None)
                return r

            def exec_step(st):
                r, kind, emit, cost = st.pop(0)
                if isinstance(r, tuple):
                    r = at_done.get((r[1], r[2]), 0.0)
                if kind == "mm":
                    clk["pe"] = max(clk["pe"], r) + cost
                    emit()
                elif kind == "sc":
                    clk["sc"] = max(clk["sc"], clk["pe"]) + cost
                    emit()
                elif kind == "none":
                    emit()
                else:
                    clk["dve"] = max(clk["dve"], clk["pe"]) + cost
                    emit()
                if not st:
                    active.remove(st)
                    refill_active()

            def pump_fills(target, allow_drain=True):
                refill_active()
                while clk["pe"] + MM <= target:
                    pick = None
                    for st in active:
                        rd = step_ready(st)
                        if rd is None:
                            continue
                        if st[0][1] in ("dve", "sc", "none"):
                            if allow_drain:
                                pick = st
                                break
                            continue
                        if rd <= max(clk["pe"] + 120.0, target - MM):
                            pick = st
                            break
                    if pick is None:
                        if warm_ok[0] and warm_cnt[0] < 90 and active:
                            nxt = min(
                                (step_ready(st) for st in active if step_ready(st) is not None),
                                default=None,
                            )
                            if nxt is None:
                                return
                            while clk["pe"] + MM < min(nxt, target) and warm_cnt[0] < 90:
                                emit_warm()
                                clk["pe"] = clk["pe"] + MM
                            if clk["pe"] + MM > target:
                                return
                            continue
                        return
                    exec_step(pick)

            def ensure(pred):
                # force-run streams in order until pred() holds
                guard = 0
                while not pred() and guard < 10000:
                    guard += 1
                    refill_active()
                    picked = False
                    for st in active:
                        rd = step_ready(st)
                        if rd is not None:
                            exec_step(st)
                            picked = True
                            break
                    if not picked:
                        if active:
                            exec_step(active[0])
                        else:
                            break

            # ---- attention ----------------------------------------------
            slot_ring = []     # global exp-end ring (s pool bufs=2)
            o_free = [clk["pe"]]
            deferred = []      # deferred recip ops (DVE)

            def emit_chunk(pair, ci):
                njt = 4 * (ci + 1)
                ensure(lambda: (pair, ci) in qt_done)
                for q in range(ci + 1):
                    ensure(lambda q=q: (pair, q) in kt_done)
                ensure(lambda: njt - 1 in v_done)
                pt_tiles = {}

                def emit_S(jt):
                    rd = max(
                        qt_done.get((pair, ci), 0.0),
                        kt_done.get((pair, jt // 4), 0.0),
                        v_done.get(jt, 0.0),
                    )
                    if len(slot_ring) >= 2:
                        rd = max(rd, slot_ring[-2])
                    r = jt - 4 * ci
                    pump_fills(rd, allow_drain=(r < -1 or jt < 2))
                    st = ps_pool.tile([128, 2, 512], f32, tag="s")
                    i0 = max(0, 128 * r)
                    for s in range(2):
                        nc.tensor.matmul(
                            st[:, s, i0:512],
                            lhsT=KT_sb[64 * s : 64 * (s + 1), pair, jt * 128 : (jt + 1) * 128],
                            rhs=QT_sb[64 * s : 64 * (s + 1), pair, ci * 512 + i0 : (ci + 1) * 512],
                            start=True,
                            stop=True,
                            skip_group_check=True,
                        )
                    clk["pe"] = max(clk["pe"], rd) + 320.0
                    pt = p_pool.tile([128, 2, 512], bf, tag="p")
                    nc.scalar.activation(pt[:, :, i0:512], st[:, :, i0:512], Exp, scale=0.125)
                    ap = 2 * (512 - i0)
                    clk["sc"] = max(clk["sc"], clk["pe"] + SEM) + EXP_FIX + ap * EXP_EL
                    slot_ring.append(clk["sc"])
                    if r >= 0:
                        nc.vector.tensor_mul(
                            pt[:, :, i0 : i0 + 128], pt[:, :, i0 : i0 + 128], tri_sb[:]
                        )
                        clk["dve"] = max(clk["dve"], clk["sc"] + SEM) + MASK_C
                        pt_ready = clk["dve"] + SEM
                    else:
                        pt_ready = clk["sc"] + SEM
                    pt_tiles[jt] = (pt, i0, pt_ready)
                    if jt == 2 and deferred:
                        for f in deferred:
                            f()
                        deferred.clear()

                def emit_PV(jt):
                    warm_ok[0] = False
                    pt, i0, rdy = pt_tiles.pop(jt)
                    if jt == 0:
                        rdy = max(rdy, o_free[0])
                    pump_fills(rdy, allow_drain=(jt < 4 * ci - 1))
                    for s, ot in enumerate((o0, o1)):
                        nc.tensor.matmul(
                            ot[:, i0:512],
                            lhsT=V_sb[:, jt, 2 * pair + s, :],
                            rhs=pt[:, s, i0:512],
                            start=(jt == 0),
                            stop=(jt == njt - 1),
                            skip_group_check=True,
                        )
                    clk["pe"] = max(clk["pe"], rdy) + 2 * 240.0

                emit_S(0)
                for jt in range(1, njt):
                    emit_S(jt)
                    emit_PV(jt - 1)
                emit_PV(njt - 1)

                # ---- normalize ----
                oc0 = oc_pool.tile([128, 512], f32, tag="oc0")
                oc1 = oc_pool.tile([128, 512], f32, tag="oc1")
                dn0 = rb_pool.tile([1, 512], f32, tag="dn0")
                dn1 = rb_pool.tile([1, 512], f32, tag="dn1")
                nc.vector.tensor_copy(oc0[0:D, :], o0[0:D, :])
                nc.vector.tensor_copy(dn0[:], o0[D : D + 1, :])
                if (pair == NPAIR - 1 and ci == NI - 1) or clk["sc"] + 1400.0 < clk["pe"] - 900.0:
                    nc.scalar.copy(oc1[D : 2 * D, :], o1[0:D, :])
                    nc.scalar.copy(dn1[:], o1[D : D + 1, :])
                    clk["sc"] = max(clk["sc"], clk["pe"]) + 1300.0
                    clk["dve"] = max(clk["dve"], clk["pe"] + SEM) + OC_C + RECIP_C
                    o_free[0] = max(clk["dve"], clk["sc"])
                else:
                    nc.vector.tensor_copy(oc1[D : 2 * D, :], o1[0:D, :])
                    nc.vector.tensor_copy(dn1[:], o1[D : D + 1, :])
                    clk["dve"] = max(clk["dve"], clk["pe"] + SEM) + 2 * OC_C + 2 * RECIP_C
                    o_free[0] = clk["dve"]

                last = pair == NPAIR - 1 and ci == NI - 1

                def norm_tail(pair=pair, ci=ci, oc0=oc0, oc1=oc1, dn0=dn0, dn1=dn1, last=last):
                    rc = rb_pool.tile([1, 1024], f32, tag="rc")
                    rb = rb_pool.tile([128, 1024], f32, tag="rb")
                    if last:
                        # per-head pipeline: head0 mul overlaps head1 chain
                        nc.vector.reciprocal_approx_fast(rc[0:1, 0:512], dn0[:])
                        nc.gpsimd.partition_broadcast(rb[:, 0:512], rc[0:1, 0:512])
                        nc.vector.reciprocal_approx_fast(rc[0:1, 512:1024], dn1[:])
                        nc.gpsimd.partition_broadcast(rb[:, 512:1024], rc[0:1, 512:1024])
                        nc.vector.tensor_mul(
                            AT_sb[0:D, pair, ci * 512 : (ci + 1) * 512],
                            oc0[0:D, :],
                            rb[0:D, 0:512],
                        )
                        nc.vector.tensor_mul(
                            AT_sb[D : 2 * D, pair, ci * 512 : (ci + 1) * 512],
                            oc1[D : 2 * D, :],
                            rb[D : 2 * D, 512:1024],
                        )
                        clk["dve"] = max(clk["dve"], clk["pool"]) + 2 * RECIP_C + 2 * 900.0
                        at_done[(pair, ci)] = clk["dve"] + SEM
                        return
                    nc.vector.reciprocal_approx_fast(rc[0:1, 0:512], dn0[:])
                    nc.vector.reciprocal_approx_fast(rc[0:1, 512:1024], dn1[:])
                    clk["dve"] = clk["dve"] + 2 * RECIP_C
                    nc.gpsimd.partition_broadcast(rb[:], rc[:])
                    clk["pool"] = max(clk["pool"], clk["dve"] + SEM) + 2 * BCAST_C
                    nc.vector.tensor_mul(
                        AT_sb[0:D, pair, ci * 512 : (ci + 1) * 512],
                        oc0[0:D, :],
                        rb[0:D, 0:512],
                    )
                    nc.vector.tensor_mul(
                        AT_sb[D : 2 * D, pair, ci * 512 : (ci + 1) * 512],
                        oc1[D : 2 * D, :],
                        rb[D : 2 * D, 512:1024],
                    )
                    clk["dve"] = max(clk["dve"], clk["pool"] + SEM) + 2 * 900.0
                    at_done[(pair, ci)] = clk["dve"] + SEM

                deferred.append(norm_tail)

            for ci in range(NI):
                for pair in range(NPAIR):
                    emit_chunk(pair, ci)
            for f in deferred:
                f()
            deferred.clear()

            # ---- tail: drain all remaining fills ----
            attn_done[0] = True
            max_active[0] = 6
            refill_active()
            guard = 0
            while (active or streams) and guard < 20000:
                guard += 1
                refill_active()
                picked = False
                for st in active:
                    if step_ready(st) is not None:
                        exec_step(st)
                        picked = True
                        break
                if not picked and active:
                    exec_step(active[0])
    return nc


def _get_compiled(bias_zero):
    if bias_zero not in _compiled:
        from concourse import bacc

        nc = bacc.Bacc(
            "TRN2", target_bir_lowering=False, debug=False, num_devices=N_CORES
        )
        _build(nc, bias_zero)
        nc.compile()
        _compiled[bias_zero] = nc
    return _compiled[bias_zero]


def _shard_inputs(x, w_qkv, b_qkv, w_proj):
    """Build the 8 per-core input dicts (host-side transpose/slice/cast)."""
    in_maps = []
    wq_f, wk_f, wv_f = w_qkv[:, :C], w_qkv[:, C : 2 * C], w_qkv[:, 2 * C :]
    for c in range(N_CORES):
        b, g = c // 2, c % 2
        sl = slice(g * CL, (g + 1) * CL)
        bqs = np.ascontiguousarray(b_qkv[0 * C :][sl].reshape(NPAIR, 128).T)
        bks = np.ascontiguousarray(b_qkv[1 * C :][sl].reshape(NPAIR, 128).T)
        bvs = np.ascontiguousarray(b_qkv[2 * C :][sl][None, :])
        wq_p = np.ascontiguousarray(
            wq_f[:, sl].reshape(CCH, 128, NPAIR, 128).transpose(2, 1, 0, 3)
            .reshape(NPAIR, 128, CCH * 128)
        )
        wk_p = np.ascontiguousarray(
            wk_f[:, sl].reshape(CCH, 128, NPAIR, 128).transpose(2, 1, 0, 3)
            .reshape(NPAIR, 128, CCH * 128)
        )
        xT_p = np.ascontiguousarray(
            x[b].T.reshape(CCH, 128, 4, 512).transpose(1, 2, 0, 3)
            .reshape(128, 4 * CCH * 512)
        )
        wv_p = np.ascontiguousarray(
            wv_f[:, sl].reshape(CCH, 128, CL).transpose(1, 0, 2)
            .reshape(128, CCH * CL)
        )
        in_maps.append(
            {
                "xT": xT_p.astype(BF16),
                "wq": wq_p.astype(BF16),
                "wk": wk_p.astype(BF16),
                "wv": wv_p.astype(BF16),
                "bq": bqs.astype(np.float32),
                "bk": bks.astype(np.float32),
                "bv": bvs.astype(np.float32),
                "wp": np.ascontiguousarray(w_proj[sl, :]).astype(BF16),
            }
        )
    return in_maps


def kernel(x, w_qkv, b_qkv, w_proj, b_proj, _trace=False, _tmpdir=None):
    from concourse.bass_utils import run_bass_kernel_spmd

    x = np.asarray(x, dtype=np.float32)
    w_qkv = np.asarray(w_qkv, dtype=np.float32)
    b_qkv = np.asarray(b_qkv, dtype=np.float32)
    w_proj = np.asarray(w_proj, dtype=np.float32)
    b_proj = np.asarray(b_proj, dtype=np.float32)

    nc = _get_compiled(not b_qkv.any())
    in_maps = _shard_inputs(x, w_qkv, b_qkv, w_proj)
    res = run_bass_kernel_spmd(
        nc,
        in_maps,
        core_ids=list(range(N_CORES)),
        trace=_trace,
        tmpdir=_tmpdir,
    )
    out = np.empty((B, T, C), dtype=np.float32)
    for b in range(B):
        out[b] = (
            res.results[2 * b]["out"].astype(np.float32)
            + res.results[2 * b + 1]["out"].astype(np.float32)
            + b_proj
        )
    kernel._last_result = res
    return out
